# revision 2
# baseline (speedup 1.0000x reference)
"""CanonGLU feedforward layer on 8 TRN2 NeuronCores — fp8 DoubleRow,
split-operand (hi+lo) version.

Math (per reference):
    gate = x @ w_gate.T ; up = x @ w_up.T            # [B,T,F]
    gate += causal_dconv(gate, conv_w[:F]) ; up += causal_dconv(up, conv_w[F:])
    out  = (up * silu(gate)) @ w_down.T              # [B,T,D]

Sharding: tensor-parallel over d_ff. Core c owns f-slice [c*1024,(c+1)*1024)
of w_gate/w_up/conv_w (column parallel) and w_down (row parallel); x
replicated.  Each core computes a full-shape partial output; the host sums
the 8 partials (the "all-reduce").

Precision scheme: e4m3 DoubleRow matmuls run at 0.5 cyc per output column
with K=256 per instruction — 4x the fp16 rate.  A single e4m3 quantization
of ANY matmul operand exceeds the 2e-2 gate (measured 2.6-3.2e-2 each), so
every operand is split v ~ hi + lo with hi = e4m3(v*S) and lo = e4m3(v*S -
hi) (same scale — no 16x mismatch in the psum), and each matmul becomes
three DoubleRow groups accumulating into one psum:
    x_hi@w_hi + x_hi@w_lo + x_lo@w_hi        (lo*lo term ~2^-8 rel, dropped)
That is 1.5 fp16-equivalents of PE time per matmul = 75% of the fp16
kernel's PE cost, at ~fp16 precision (measured end-to-end 1.08e-2).

The depthwise conv runs ON THE PE as diagonal-stationary DoubleRow matmuls
accumulating into the projection psum: tap k needs c_k[f]*gate[f, t+k-3] =
diag(c_k) @ shift_k(gate8), built from a single-e4m3 copy of the gate
(tap terms are ~10% magnitude; their quantization is in the 1.08e-2 total).

Scales (e4m3 range [2^-9, 240]):
    x*SX (SX=8), w_gate/w_up*SW (SW=512) -> proj psum scale SPS = 4096
    gate8 copy *SG8 (16) -> |gate|<=7.6 -> 122; diag c*SPS/SG8 = c*256 <= 116
    h*SH (SH=4, |h|<=35 -> 140), w_down*SWD (512) -> out psum scale 2048

Engine budget per core (64 (fc,tile) units, TimelineSim cost model):
    PE   ~527us  (mains 2x24 DR + taps 2x2 DR per unit; down 12 DR x 16dc)
    ACT  ~196us  (gate evac, silu, h_hi/h_lo casts, half the down evacs)
    DVE  ~165us  (up evac, h16, h residual, half the down evacs)
    Pool — GPSIMD cannot access PSUM (walrus birverifier); idle.
    DMA  ~130us  (in 29.4MB fp8 hi+lo, out 16.8MB fp16 partials)
"""

import numpy as np
import ml_dtypes

import concourse.bass as bass
import concourse.mybir as mybir
import concourse.tile as tile
from concourse import bacc
from concourse.bass_utils import run_bass_kernel_spmd

F16 = mybir.dt.float16
F32 = mybir.dt.float32
F8 = mybir.dt.float8e4
E4NP = ml_dtypes.float8_e4m3
AF = mybir.ActivationFunctionType
ALU = mybir.AluOpType
DR = mybir.MatmulPerfMode.DoubleRow

B, T, D, F = 2, 2048, 2048, 8192
NCORES = 8
FC_PER_CORE = F // NCORES          # 1024 f per core
TT = B * T                         # 4096 tokens total
NT = 512                           # token tile (one PSUM bank of fp32)
N_TILES = TT // NT                 # 8
TILES_PER_BATCH = T // NT          # 4 (conv halo resets at 0 and 4)
DC = D // 128                      # 16 d-chunks
FC = FC_PER_CORE // 128            # 8 f-chunks per core
GROW = NT + 4                      # conv fp8 row: 3 halo + 512 data + pad
# (pad keeps the slot stride EVEN — an odd-stride DoubleRow moving
# operand crashes the hardware fetch)

SX = 8.0                           # x fp8 scale
SW = 512.0                         # w_gate/w_up fp8 scale
SPS = SX * SW                      # gate/up psum scale (4096)
SG8 = 16.0                         # gate/up fp8 copy scale
SH = 4.0                           # h fp8 scale (|h| <= ~35)
SWD = 512.0                        # w_down fp8 scale
SOUT = SH * SWD                    # down psum scale (2048)


def build_nc():
    nc = bacc.Bacc(None, target_bir_lowering=False, debug=False)

    xhT = nc.dram_tensor("xhT", [D, TT], F8, kind="ExternalInput")
    xlT = nc.dram_tensor("xlT", [D, TT], F8, kind="ExternalInput")
    wghT = nc.dram_tensor("wghT", [D, FC_PER_CORE], F8, kind="ExternalInput")
    wglT = nc.dram_tensor("wglT", [D, FC_PER_CORE], F8, kind="ExternalInput")
    wuhT = nc.dram_tensor("wuhT", [D, FC_PER_CORE], F8, kind="ExternalInput")
    wulT = nc.dram_tensor("wulT", [D, FC_PER_CORE], F8, kind="ExternalInput")
    wdhT = nc.dram_tensor("wdhT", [FC_PER_CORE, D], F8, kind="ExternalInput")
    wdlT = nc.dram_tensor("wdlT", [FC_PER_CORE, D], F8, kind="ExternalInput")
    # compact conv taps [p, fc, branch, k] and a 128x128 identity mask; the
    # diagonal stationary tiles are built on-device (idle DVE at startup).
    cw = nc.dram_tensor("cw", [128, FC, 2, 4], F32, kind="ExternalInput")
    eye = nc.dram_tensor("eye", [128, 128], F16, kind="ExternalInput")
    outT = nc.dram_tensor("outT", [D, TT], F16, kind="ExternalOutput")

    def rearr(t, pat):
        return t.rearrange(pat, p=128)

    xhTr = rearr(xhT, "(dc p) t -> p dc t")
    xlTr = rearr(xlT, "(dc p) t -> p dc t")
    wgTr = [rearr(wghT, "(dc p) f -> p dc f"), rearr(wglT, "(dc p) f -> p dc f")]
    wuTr = [rearr(wuhT, "(dc p) f -> p dc f"), rearr(wulT, "(dc p) f -> p dc f")]
    wdTr = [rearr(wdhT, "(fc p) d -> p fc d"), rearr(wdlT, "(fc p) d -> p fc d")]
    outTr = rearr(outT, "(dp p) t -> p dp t")

    with tile.TileContext(nc) as tc:
        with (
            tc.tile_pool(name="consts", bufs=1) as consts,
            tc.tile_pool(name="xp", bufs=2) as xpool,
            tc.tile_pool(name="gb", bufs=2) as gbpool,
            tc.tile_pool(name="ub", bufs=2) as ubpool,
            tc.tile_pool(name="hp", bufs=2) as hpool,
            tc.tile_pool(name="sgp", bufs=9) as sgpool,
            tc.tile_pool(name="scr", bufs=2) as scrpool,
            tc.tile_pool(name="op", bufs=3) as opool,
            tc.tile_pool(name="psg", bufs=3, space="PSUM") as ps_g,
            tc.tile_pool(name="psu", bufs=2, space="PSUM") as ps_u,
            tc.tile_pool(name="pso", bufs=3, space="PSUM") as ps_o,
        ):
            wg_sb = consts.tile([128, 2, DC, FC_PER_CORE], F8)
            wu_sb = consts.tile([128, 2, DC, FC_PER_CORE], F8)
            wd_sb = consts.tile([128, 2, FC, D], F8)
            cd_sb = consts.tile([128, FC, 2, 2, 2, 128], F8)
            cw_sb = consts.tile([128, FC, 2, 4], F32)
            eye_sb = consts.tile([128, 128], F16)

            x_tiles = {}

            def load_x(tt):
                # one DMA per (tile, half): HWDGE descriptor-gen holds ~627ns
                # per dma_start, so fine-grained loads serialize on it
                x_t = xpool.tile([128, 2, DC, NT], F8)
                nc.sync.dma_start(out=x_t[:, 0], in_=xhTr[:, :, bass.ts(tt, NT)])
                nc.sync.dma_start(out=x_t[:, 1], in_=xlTr[:, :, bass.ts(tt, NT)])
                x_tiles[tt] = x_t

            # PE warmup: dummy matmuls on zeroed SBUF fill the startup DMA
            # wait (w_gate hi+lo 4.2MB + x0 2MB ~ 17us) so the PE p-state
            # reaches (and keeps) 2.4 GHz before the first real matmul.
            warm_sb = consts.tile([128, NT], F16)
            nc.gpsimd.memset(warm_sb[:], 0.0)
            warm_ps = ps_g.tile([128, NT], F32, name="psg")
            for _ in range(34):
                nc.tensor.matmul(
                    warm_ps[:], warm_sb[:, 0:128], warm_sb[:],
                    start=True, stop=True)

            # startup DMA order = first-need order for the tile-0 gate-first
            # schedule.
            nc.sync.dma_start(out=eye_sb[:], in_=eye[:])
            nc.sync.dma_start(out=cw_sb[:], in_=cw[:])
            nc.sync.dma_start(out=wg_sb[:, 0], in_=wgTr[0][:])
            nc.sync.dma_start(out=wg_sb[:, 1], in_=wgTr[1][:])
            load_x(0)
            nc.sync.dma_start(out=wu_sb[:, 0], in_=wuTr[0][:])
            nc.sync.dma_start(out=wu_sb[:, 1], in_=wuTr[1][:])
            load_x(1)
            nc.sync.dma_start(out=wd_sb[:, 0], in_=wdTr[0][:])
            nc.sync.dma_start(out=wd_sb[:, 1], in_=wdTr[1][:])
            # build the diagonal tap stationaries on the (startup-idle) DVE:
            # cd[:, fc, br, pr, i, :] = eye * c_{2pr+i}[partition]
            for fc in range(FC):
                for br in range(2):
                    for k in range(4):
                        nc.vector.tensor_scalar(
                            cd_sb[:, fc, br, k // 2, k % 2, :], eye_sb[:],
                            cw_sb[:, fc, br, k:k + 1], None, ALU.mult)

            g_prev = u_prev = None
            h_tiles = {}

            def emit_main(w_sb, psp, x_t, k):
                # three DoubleRow groups: xh@wh, xh@wl, xl@wh (same psum
                # scale: lo parts are stored at the hi scale, not 16x)
                first = True
                for wi, xi in ((0, 0), (1, 0), (0, 1)):
                    for i in range(DC // 2):
                        nc.tensor.matmul(
                            psp[:],
                            w_sb[:, wi, 2 * i:2 * i + 2, bass.ts(k, 128)],
                            x_t[:, xi, 2 * i:2 * i + 2, :],
                            start=first, stop=False, perf_mode=DR,
                            skip_group_check=not first)
                        first = False

            def emit_taps(buf, k, psp, br):
                # conv taps on the PE: diag(c_k)@shift_k accumulated into the
                # projection psum (residual term is already there).  buf slot
                # 0 holds gate8, slot 1 the 1-token-shifted copy, so the
                # DoubleRow pair (shift 2pr, 2pr+1) is the non-overlapping
                # strided view [p, 2, NT] at offset 2pr — an overlapping AP
                # on the moving operand kills the hardware fetch.
                for pr in range(2):
                    nc.tensor.matmul(
                        psp[:],
                        cd_sb[:, k, br, pr, :, :],
                        buf[:, k, :, 2 * pr:2 * pr + NT],
                        start=False, stop=(pr == 1),
                        perf_mode=DR, skip_group_check=True)

            def emit_silu(psg):
                sg = sgpool.tile([128, NT], F16)
                nc.scalar.activation(
                    out=sg[:], in_=psg[:], func=AF.Silu, scale=1.0 / SPS)
                return sg

            def emit_h(h_t, k, psu, sg):
                # h*SH = (psum_u * SH/SPS) * sg, split hi (fp8) + lo (fp8 of
                # the fp16 residual) for the split down-proj
                h16 = scrpool.tile([128, NT], F16, name="h16")
                nc.vector.scalar_tensor_tensor(
                    out=h16[:], in0=psu[:], scalar=SH / SPS,
                    in1=sg[:], op0=ALU.mult, op1=ALU.mult)
                nc.scalar.activation(
                    out=h_t[:, 0, k, :], in_=h16[:], func=AF.Copy)
                r16 = scrpool.tile([128, NT], F16, name="r16")
                nc.vector.tensor_tensor(
                    out=r16[:], in0=h16[:], in1=h_t[:, 0, k, :], op=ALU.subtract)
                nc.scalar.activation(
                    out=h_t[:, 1, k, :], in_=r16[:], func=AF.Copy)

            def down_pair(tt, k, last=False):
                """Down-proj for dc = 2k, 2k+1: three DoubleRow groups per
                psum (hh, hl, lh), evacs alternated ACT/DVE (GPSIMD cannot
                read PSUM), one out-DMA per pair."""
                h_t = h_tiles[tt]
                o_sb = opool.tile([128, 2, NT], F16)
                for half in range(2):
                    dc = 2 * k + half
                    pool, tag = ((ps_o, "pso"), (ps_g, "psg"),
                                 (ps_u, "psu"))[dc % 3 if last else 0]
                    pso = pool.tile([128, NT], F32, name=tag)
                    first = True
                    for wi, hi in ((0, 0), (1, 0), (0, 1)):
                        for j in range(FC // 2):
                            nc.tensor.matmul(
                                pso[:],
                                wd_sb[:, wi, 2 * j:2 * j + 2, bass.ts(dc, 128)],
                                h_t[:, hi, 2 * j:2 * j + 2, :],
                                start=first,
                                stop=(wi == 0 and hi == 1 and j == FC // 2 - 1),
                                perf_mode=DR, skip_group_check=not first)
                            first = False
                    o_slot = o_sb[:, half, :]
                    if dc % 2 == 0:
                        nc.scalar.activation(
                            out=o_slot, in_=pso[:], func=AF.Copy,
                            scale=1.0 / SOUT)
                    else:
                        nc.vector.tensor_scalar(
                            o_slot, pso[:], 1.0 / SOUT, None, ALU.mult)
                eng = nc.sync if k % 2 == 0 else nc.scalar
                eng.dma_start(
                    out=outTr[:, 2 * k:2 * k + 2, bass.ts(tt, NT)],
                    in_=o_sb[:])

            def new_bufs(tt):
                # slot 0: gate8/up8 at token offset -3 (3 halo + 512 data);
                # slot 1: the same data shifted one token (2 halo + 512 data)
                g_cur = gbpool.tile([128, FC, 2, GROW], F8)
                u_cur = ubpool.tile([128, FC, 2, GROW], F8)
                h_t = hpool.tile([128, 2, FC, NT], F8)
                # conv halo: last tokens of the previous tile's fp8 copies
                # (zeros at the start of each batch — causal left pad).
                for br, (buf, prev) in enumerate(((g_cur, g_prev),
                                                  (u_cur, u_prev))):
                    if tt % TILES_PER_BATCH == 0:
                        nc.vector.memset(buf[:, :, 0, 0:3], 0.0)
                        nc.vector.memset(buf[:, :, 1, 0:2], 0.0)
                    else:
                        nc.vector.tensor_copy(
                            out=buf[:, :, 0, 0:3],
                            in_=prev[:, :, 0, NT:NT + 3])
                        nc.vector.tensor_copy(
                            out=buf[:, :, 1, 0:2],
                            in_=prev[:, :, 1, NT:NT + 2])
                return g_cur, u_cur, h_t

            def tile0_phase():
                """Tile 0 streams gate first (only w_gate + x(0) must have
                landed), then up — hides the w_up DMA behind the gate pass
                instead of stalling the PE."""
                nonlocal g_prev, u_prev
                x_t = x_tiles.pop(0)
                g_cur, u_cur, h_t = new_bufs(0)
                sgs = {}
                # gate taps lag TWO mains so the first tap (which needs the
                # x_lo DMA-built diags) never stalls the in-order PE queue
                gq = []
                for k in range(FC):
                    if len(gq) == 2:
                        kk, psg_old = gq.pop(0)
                        emit_taps(g_cur, kk, psg_old, 0)
                        sgs[kk] = emit_silu(psg_old)
                    psg = ps_g.tile([128, NT], F32)
                    emit_main(wg_sb, psg, x_t, k)
                    nc.scalar.activation(
                        out=g_cur[:, k, 0, 3:3 + NT], in_=psg[:],
                        func=AF.Copy, scale=SG8 / SPS)
                    nc.vector.tensor_scalar(
                        g_cur[:, k, 1, 2:2 + NT], psg[:], SG8 / SPS, None,
                        ALU.mult)
                    gq.append((k, psg))
                for kk, psg_old in gq:
                    emit_taps(g_cur, kk, psg_old, 0)
                    sgs[kk] = emit_silu(psg_old)
                upend = None
                for k in range(FC):
                    if upend is not None:
                        emit_taps(u_cur, k - 1, upend, 1)
                        emit_h(h_t, k - 1, upend, sgs[k - 1])
                    psu = ps_u.tile([128, NT], F32)
                    emit_main(wu_sb, psu, x_t, k)
                    nc.vector.tensor_scalar(
                        u_cur[:, k, 0, 3:3 + NT], psu[:], SG8 / SPS, None,
                        ALU.mult)
                    nc.scalar.activation(
                        out=u_cur[:, k, 1, 2:2 + NT], in_=psu[:],
                        func=AF.Copy, scale=SG8 / SPS)
                    upend = psu
                emit_taps(u_cur, FC - 1, upend, 1)
                emit_h(h_t, FC - 1, upend, sgs[FC - 1])
                g_prev, u_prev = g_cur, u_cur
                h_tiles[0] = h_t

            # pending (g_cur, u_cur, h_t, fc, psg, psu): projections whose
            # conv taps + silu + h are emitted one fc later, so the in-order
            # PE queue never waits on the evac->tap dependency.
            pend = [None]

            def tile_phase(tt):
                """Emit gate/up(tt) interleaved with down(tt-1)."""
                nonlocal g_prev, u_prev
                if tt < N_TILES:
                    x_t = x_tiles.pop(tt)
                    g_cur, u_cur, h_t = new_bufs(tt)
                for k in range(FC):
                    prev = pend[0]
                    pend[0] = None
                    if prev is not None:
                        # PE: taps for the previous unit first (evacs landed
                        # last iteration)
                        emit_taps(prev[0], prev[3], prev[4], 0)
                        emit_taps(prev[1], prev[3], prev[5], 1)
                    if tt < N_TILES:
                        psg = ps_g.tile([128, NT], F32)
                        psu = ps_u.tile([128, NT], F32)
                        emit_main(wg_sb, psg, x_t, k)
                        # fp8 copies (and their 1-token-shifted twins) of the
                        # pre-conv projections for the taps and next halo;
                        # split across ACT and DVE per branch
                        nc.scalar.activation(
                            out=g_cur[:, k, 0, 3:3 + NT], in_=psg[:],
                            func=AF.Copy, scale=SG8 / SPS)
                        nc.vector.tensor_scalar(
                            g_cur[:, k, 1, 2:2 + NT], psg[:], SG8 / SPS, None,
                            ALU.mult)
                        emit_main(wu_sb, psu, x_t, k)
                        nc.vector.tensor_scalar(
                            u_cur[:, k, 0, 3:3 + NT], psu[:], SG8 / SPS, None,
                            ALU.mult)
                        nc.scalar.activation(
                            out=u_cur[:, k, 1, 2:2 + NT], in_=psu[:],
                            func=AF.Copy, scale=SG8 / SPS)
                    if prev is not None:
                        # silu + h for the previous unit, after this unit's
                        # evacs in the ACT/DVE queues (shortens the
                        # PE-critical evac->tap chain)
                        sg = emit_silu(prev[4])
                        emit_h(prev[2], prev[3], prev[5], sg)
                    if tt < N_TILES:
                        pend[0] = (g_cur, u_cur, h_t, k, psg, psu)
                    if tt >= 1:
                        down_pair(tt - 1, k, last=(tt == N_TILES))
                if tt >= 1:
                    h_tiles.pop(tt - 1)
                if tt < N_TILES:
                    g_prev, u_prev = g_cur, u_cur
                    h_tiles[tt] = h_t

            tile0_phase()
            for tt in range(1, N_TILES + 1):
                if tt + 1 <= N_TILES - 1:
                    load_x(tt + 1)
                tile_phase(tt)

    nc.compile()
    return nc


_NC_CACHE = None


def _get_nc():
    global _NC_CACHE
    if _NC_CACHE is None:
        _NC_CACHE = build_nc()
    return _NC_CACHE


def _split8(a, scale):
    s = np.asarray(a * np.float32(scale), dtype=np.float32)
    hi = s.astype(E4NP)
    lo = (s - hi.astype(np.float32)).astype(E4NP)
    return hi, lo


def _prep_inputs(x, w_gate, w_up, w_down, conv_w):
    xh, xl = _split8(np.ascontiguousarray(x.reshape(TT, D).T), SX)  # [D, TT]
    # compact conv taps [p, fc, branch, k], scaled for the diag stationaries
    cwf = conv_w.reshape(2, NCORES, FC, 128, 4)                 # [br,c,fc,p,k]
    eye = np.eye(128, dtype=np.float16)
    in_maps = []
    for c in range(NCORES):
        fs = slice(c * FC_PER_CORE, (c + 1) * FC_PER_CORE)
        wgh, wgl = _split8(np.ascontiguousarray(w_gate[fs].T), SW)
        wuh, wul = _split8(np.ascontiguousarray(w_up[fs].T), SW)
        wdh, wdl = _split8(np.ascontiguousarray(w_down[:, fs].T), SWD)
        cwc = np.ascontiguousarray(
            cwf[:, c].transpose(2, 1, 0, 3)) * (SPS / SG8)      # [p,fc,br,k]
        in_maps.append({
            "xhT": xh, "xlT": xl, "wghT": wgh, "wglT": wgl,
            "wuhT": wuh, "wulT": wul, "wdhT": wdh, "wdlT": wdl,
            "cw": cwc.astype(np.float32), "eye": eye})
    return in_maps


def run_spmd(in_maps, **kwargs):
    nc = _get_nc()
    return run_bass_kernel_spmd(
        nc, in_maps, core_ids=list(range(NCORES)), **kwargs)


def kernel(x, w_gate, w_up, w_down, conv_w):
    in_maps = _prep_inputs(
        np.asarray(x, dtype=np.float32), np.asarray(w_gate, dtype=np.float32),
        np.asarray(w_up, dtype=np.float32),
        np.asarray(w_down, dtype=np.float32),
        np.asarray(conv_w, dtype=np.float32))
    res = run_spmd(in_maps)
    acc = np.zeros((D, TT), np.float32)
    for r in res.results:
        acc += r["outT"].astype(np.float32)
    return np.ascontiguousarray(acc.T).reshape(B, T, D)


# revision 3
# speedup vs baseline: 1.0008x; 1.0008x over previous
"""CanonGLU feedforward layer on 8 TRN2 NeuronCores — fp8 DoubleRow,
split-operand (hi+lo) version.

Math (per reference):
    gate = x @ w_gate.T ; up = x @ w_up.T            # [B,T,F]
    gate += causal_dconv(gate, conv_w[:F]) ; up += causal_dconv(up, conv_w[F:])
    out  = (up * silu(gate)) @ w_down.T              # [B,T,D]

Sharding: tensor-parallel over d_ff. Core c owns f-slice [c*1024,(c+1)*1024)
of w_gate/w_up/conv_w (column parallel) and w_down (row parallel); x
replicated.  Each core computes a full-shape partial output; the host sums
the 8 partials (the "all-reduce").

Precision scheme: e4m3 DoubleRow matmuls run at 0.5 cyc per output column
with K=256 per instruction — 4x the fp16 rate.  A single e4m3 quantization
of ANY matmul operand exceeds the 2e-2 gate (measured 2.6-3.2e-2 each), so
every operand is split v ~ hi + lo with hi = e4m3(v*S) and lo = e4m3(v*S -
hi) (same scale — no 16x mismatch in the psum), and each matmul becomes
three DoubleRow groups accumulating into one psum:
    x_hi@w_hi + x_hi@w_lo + x_lo@w_hi        (lo*lo term ~2^-8 rel, dropped)
That is 1.5 fp16-equivalents of PE time per matmul = 75% of the fp16
kernel's PE cost, at ~fp16 precision (measured end-to-end 1.08e-2).

The depthwise conv runs ON THE PE as diagonal-stationary DoubleRow matmuls
accumulating into the projection psum: tap k needs c_k[f]*gate[f, t+k-3] =
diag(c_k) @ shift_k(gate8), built from a single-e4m3 copy of the gate
(tap terms are ~10% magnitude; their quantization is in the 1.08e-2 total).

Scales (e4m3 range [2^-9, 240]):
    x*SX (SX=8), w_gate/w_up*SW (SW=512) -> proj psum scale SPS = 4096
    gate8 copy *SG8 (16) -> |gate|<=7.6 -> 122; diag c*SPS/SG8 = c*256 <= 116
    h*SH (SH=4, |h|<=35 -> 140), w_down*SWD (512) -> out psum scale 2048

Engine budget per core (64 (fc,tile) units, TimelineSim cost model):
    PE   ~527us  (mains 2x24 DR + taps 2x2 DR per unit; down 12 DR x 16dc)
    ACT  ~196us  (gate evac, silu, h_hi/h_lo casts, half the down evacs)
    DVE  ~165us  (up evac, h16, h residual, half the down evacs)
    Pool — GPSIMD cannot access PSUM (walrus birverifier); idle.
    DMA  ~130us  (in 29.4MB fp8 hi+lo, out 16.8MB fp16 partials)
"""

import numpy as np
import ml_dtypes

import concourse.bass as bass
import concourse.mybir as mybir
import concourse.tile as tile
from concourse import bacc
from concourse.bass_utils import run_bass_kernel_spmd

F16 = mybir.dt.float16
F32 = mybir.dt.float32
F8 = mybir.dt.float8e4
E4NP = ml_dtypes.float8_e4m3
AF = mybir.ActivationFunctionType
ALU = mybir.AluOpType
DR = mybir.MatmulPerfMode.DoubleRow

B, T, D, F = 2, 2048, 2048, 8192
NCORES = 8
FC_PER_CORE = F // NCORES          # 1024 f per core
TT = B * T                         # 4096 tokens total
NT = 512                           # token tile (one PSUM bank of fp32)
N_TILES = TT // NT                 # 8
TILES_PER_BATCH = T // NT          # 4 (conv halo resets at 0 and 4)
DC = D // 128                      # 16 d-chunks
FC = FC_PER_CORE // 128            # 8 f-chunks per core
GROW = NT + 4                      # conv fp8 row: 3 halo + 512 data + pad
# (pad keeps the slot stride EVEN — an odd-stride DoubleRow moving
# operand crashes the hardware fetch)

SX = 8.0                           # x fp8 scale
SW = 512.0                         # w_gate/w_up fp8 scale
SPS = SX * SW                      # gate/up psum scale (4096)
SG8 = 16.0                         # gate/up fp8 copy scale
SH = 4.0                           # h fp8 scale (|h| <= ~35)
SWD = 512.0                        # w_down fp8 scale
SOUT = SH * SWD                    # down psum scale (2048)


def build_nc():
    nc = bacc.Bacc(None, target_bir_lowering=False, debug=False)

    xhT = nc.dram_tensor("xhT", [D, TT], F8, kind="ExternalInput")
    xlT = nc.dram_tensor("xlT", [D, TT], F8, kind="ExternalInput")
    wghT = nc.dram_tensor("wghT", [D, FC_PER_CORE], F8, kind="ExternalInput")
    wglT = nc.dram_tensor("wglT", [D, FC_PER_CORE], F8, kind="ExternalInput")
    wuhT = nc.dram_tensor("wuhT", [D, FC_PER_CORE], F8, kind="ExternalInput")
    wulT = nc.dram_tensor("wulT", [D, FC_PER_CORE], F8, kind="ExternalInput")
    wdhT = nc.dram_tensor("wdhT", [FC_PER_CORE, D], F8, kind="ExternalInput")
    wdlT = nc.dram_tensor("wdlT", [FC_PER_CORE, D], F8, kind="ExternalInput")
    # compact conv taps [p, fc, branch, k] and a 128x128 identity mask; the
    # diagonal stationary tiles are built on-device (idle DVE at startup).
    cw = nc.dram_tensor("cw", [128, FC, 2, 4], F32, kind="ExternalInput")
    eye = nc.dram_tensor("eye", [128, 128], F16, kind="ExternalInput")
    outT = nc.dram_tensor("outT", [D, TT], F16, kind="ExternalOutput")

    def rearr(t, pat):
        return t.rearrange(pat, p=128)

    xhTr = rearr(xhT, "(dc p) t -> p dc t")
    xlTr = rearr(xlT, "(dc p) t -> p dc t")
    wgTr = [rearr(wghT, "(dc p) f -> p dc f"), rearr(wglT, "(dc p) f -> p dc f")]
    wuTr = [rearr(wuhT, "(dc p) f -> p dc f"), rearr(wulT, "(dc p) f -> p dc f")]
    wdTr = [rearr(wdhT, "(fc p) d -> p fc d"), rearr(wdlT, "(fc p) d -> p fc d")]
    outTr = rearr(outT, "(dp p) t -> p dp t")

    with tile.TileContext(nc) as tc:
        with (
            tc.tile_pool(name="consts", bufs=1) as consts,
            tc.tile_pool(name="xp", bufs=2) as xpool,
            tc.tile_pool(name="gb", bufs=2) as gbpool,
            tc.tile_pool(name="ub", bufs=2) as ubpool,
            tc.tile_pool(name="hp", bufs=2) as hpool,
            tc.tile_pool(name="sgp", bufs=9) as sgpool,
            tc.tile_pool(name="scr", bufs=2) as scrpool,
            tc.tile_pool(name="op", bufs=3) as opool,
            tc.tile_pool(name="psg", bufs=3, space="PSUM") as ps_g,
            tc.tile_pool(name="psu", bufs=2, space="PSUM") as ps_u,
            tc.tile_pool(name="pso", bufs=3, space="PSUM") as ps_o,
        ):
            wg_sb = consts.tile([128, 2, DC, FC_PER_CORE], F8)
            wu_sb = consts.tile([128, 2, DC, FC_PER_CORE], F8)
            wd_sb = consts.tile([128, 2, FC, D], F8)
            cd_sb = consts.tile([128, FC, 2, 2, 2, 128], F8)
            cw_sb = consts.tile([128, FC, 2, 4], F32)
            eye_sb = consts.tile([128, 128], F16)

            x_tiles = {}

            def load_x(tt):
                # one DMA per (tile, half): HWDGE descriptor-gen holds ~627ns
                # per dma_start, so fine-grained loads serialize on it
                x_t = xpool.tile([128, 2, DC, NT], F8)
                nc.sync.dma_start(out=x_t[:, 0], in_=xhTr[:, :, bass.ts(tt, NT)])
                nc.sync.dma_start(out=x_t[:, 1], in_=xlTr[:, :, bass.ts(tt, NT)])
                x_tiles[tt] = x_t

            # PE warmup: dummy matmuls on zeroed SBUF fill the startup DMA
            # wait (w_gate hi+lo 4.2MB + x0 2MB ~ 17us) so the PE p-state
            # reaches (and keeps) 2.4 GHz before the first real matmul.
            warm_sb = consts.tile([128, NT], F16)
            nc.gpsimd.memset(warm_sb[:], 0.0)
            warm_ps = ps_g.tile([128, NT], F32, name="psg")
            for _ in range(34):
                nc.tensor.matmul(
                    warm_ps[:], warm_sb[:, 0:128], warm_sb[:],
                    start=True, stop=True)

            # startup DMA order = first-need order for the tile-0 gate-first
            # schedule.
            nc.sync.dma_start(out=eye_sb[:], in_=eye[:])
            nc.sync.dma_start(out=cw_sb[:], in_=cw[:])
            nc.sync.dma_start(out=wg_sb[:, 0], in_=wgTr[0][:])
            nc.sync.dma_start(out=wg_sb[:, 1], in_=wgTr[1][:])
            load_x(0)
            nc.sync.dma_start(out=wu_sb[:, 0], in_=wuTr[0][:])
            nc.sync.dma_start(out=wu_sb[:, 1], in_=wuTr[1][:])
            load_x(1)
            nc.sync.dma_start(out=wd_sb[:, 0], in_=wdTr[0][:])
            nc.sync.dma_start(out=wd_sb[:, 1], in_=wdTr[1][:])
            # build the diagonal tap stationaries on the (startup-idle) DVE:
            # cd[:, fc, br, pr, i, :] = eye * c_{2pr+i}[partition]
            for fc in range(FC):
                for br in range(2):
                    for k in range(4):
                        nc.vector.tensor_scalar(
                            cd_sb[:, fc, br, k // 2, k % 2, :], eye_sb[:],
                            cw_sb[:, fc, br, k:k + 1], None, ALU.mult)

            g_prev = u_prev = None
            h_tiles = {}

            def emit_main(w_sb, psp, x_t, k):
                # three DoubleRow groups: xh@wh, xh@wl, xl@wh (same psum
                # scale: lo parts are stored at the hi scale, not 16x)
                first = True
                for wi, xi in ((0, 0), (1, 0), (0, 1)):
                    for i in range(DC // 2):
                        nc.tensor.matmul(
                            psp[:],
                            w_sb[:, wi, 2 * i:2 * i + 2, bass.ts(k, 128)],
                            x_t[:, xi, 2 * i:2 * i + 2, :],
                            start=first, stop=False, perf_mode=DR,
                            skip_group_check=not first)
                        first = False

            def emit_taps(buf, k, psp, br):
                # conv taps on the PE: diag(c_k)@shift_k accumulated into the
                # projection psum (residual term is already there).  buf slot
                # 0 holds gate8, slot 1 the 1-token-shifted copy, so the
                # DoubleRow pair (shift 2pr, 2pr+1) is the non-overlapping
                # strided view [p, 2, NT] at offset 2pr — an overlapping AP
                # on the moving operand kills the hardware fetch.
                for pr in range(2):
                    nc.tensor.matmul(
                        psp[:],
                        cd_sb[:, k, br, pr, :, :],
                        buf[:, k, :, 2 * pr:2 * pr + NT],
                        start=False, stop=(pr == 1),
                        perf_mode=DR, skip_group_check=True)

            def emit_silu(psg):
                sg = sgpool.tile([128, NT], F16)
                nc.scalar.activation(
                    out=sg[:], in_=psg[:], func=AF.Silu, scale=1.0 / SPS)
                return sg

            def emit_h(h_t, k, psu, sg):
                # h*SH = (psum_u * SH/SPS) * sg, split hi (fp8) + lo (fp8 of
                # the fp16 residual) for the split down-proj
                h16 = scrpool.tile([128, NT], F16, name="h16")
                nc.vector.scalar_tensor_tensor(
                    out=h16[:], in0=psu[:], scalar=SH / SPS,
                    in1=sg[:], op0=ALU.mult, op1=ALU.mult)
                nc.scalar.activation(
                    out=h_t[:, 0, k, :], in_=h16[:], func=AF.Copy)
                r16 = scrpool.tile([128, NT], F16, name="r16")
                nc.vector.tensor_tensor(
                    out=r16[:], in0=h16[:], in1=h_t[:, 0, k, :], op=ALU.subtract)
                nc.scalar.activation(
                    out=h_t[:, 1, k, :], in_=r16[:], func=AF.Copy)

            def down_pair(tt, k, last=False):
                """Down-proj for dc = 2k, 2k+1: three DoubleRow groups per
                psum (hh, hl, lh), evacs alternated ACT/DVE (GPSIMD cannot
                read PSUM), one out-DMA per pair."""
                h_t = h_tiles[tt]
                o_sb = opool.tile([128, 2, NT], F16)
                for half in range(2):
                    dc = 2 * k + half
                    pool, tag = ((ps_o, "pso"), (ps_g, "psg"),
                                 (ps_u, "psu"))[dc % 3 if last else 0]
                    pso = pool.tile([128, NT], F32, name=tag)
                    first = True
                    for wi, hi in ((0, 0), (1, 0), (0, 1)):
                        for j in range(FC // 2):
                            nc.tensor.matmul(
                                pso[:],
                                wd_sb[:, wi, 2 * j:2 * j + 2, bass.ts(dc, 128)],
                                h_t[:, hi, 2 * j:2 * j + 2, :],
                                start=first,
                                stop=(wi == 0 and hi == 1 and j == FC // 2 - 1),
                                perf_mode=DR, skip_group_check=not first)
                            first = False
                    o_slot = o_sb[:, half, :]
                    if dc % 2 == 0:
                        nc.scalar.activation(
                            out=o_slot, in_=pso[:], func=AF.Copy,
                            scale=1.0 / SOUT)
                    else:
                        nc.vector.tensor_scalar(
                            o_slot, pso[:], 1.0 / SOUT, None, ALU.mult)
                if last:
                    # tail drain: per-dc DMAs so the final transfer only
                    # waits its own evac
                    for half in range(2):
                        eng = nc.sync if half == 0 else nc.scalar
                        eng.dma_start(
                            out=outTr[:, 2 * k + half, bass.ts(tt, NT)],
                            in_=o_sb[:, half, :])
                else:
                    eng = nc.sync if k % 2 == 0 else nc.scalar
                    eng.dma_start(
                        out=outTr[:, 2 * k:2 * k + 2, bass.ts(tt, NT)],
                        in_=o_sb[:])

            def new_bufs(tt):
                # slot 0: gate8/up8 at token offset -3 (3 halo + 512 data);
                # slot 1: the same data shifted one token (2 halo + 512 data)
                g_cur = gbpool.tile([128, FC, 2, GROW], F8)
                u_cur = ubpool.tile([128, FC, 2, GROW], F8)
                h_t = hpool.tile([128, 2, FC, NT], F8)
                # conv halo: last tokens of the previous tile's fp8 copies
                # (zeros at the start of each batch — causal left pad).
                for br, (buf, prev) in enumerate(((g_cur, g_prev),
                                                  (u_cur, u_prev))):
                    if tt % TILES_PER_BATCH == 0:
                        nc.vector.memset(buf[:, :, 0, 0:3], 0.0)
                        nc.vector.memset(buf[:, :, 1, 0:2], 0.0)
                    else:
                        nc.vector.tensor_copy(
                            out=buf[:, :, 0, 0:3],
                            in_=prev[:, :, 0, NT:NT + 3])
                        nc.vector.tensor_copy(
                            out=buf[:, :, 1, 0:2],
                            in_=prev[:, :, 1, NT:NT + 2])
                return g_cur, u_cur, h_t

            def tile0_phase():
                """Tile 0 streams gate first (only w_gate + x(0) must have
                landed), then up — hides the w_up DMA behind the gate pass
                instead of stalling the PE."""
                nonlocal g_prev, u_prev
                x_t = x_tiles.pop(0)
                g_cur, u_cur, h_t = new_bufs(0)
                sgs = {}
                # gate taps lag TWO mains so the first tap (which needs the
                # x_lo DMA-built diags) never stalls the in-order PE queue
                gq = []
                for k in range(FC):
                    if len(gq) == 2:
                        kk, psg_old = gq.pop(0)
                        emit_taps(g_cur, kk, psg_old, 0)
                        sgs[kk] = emit_silu(psg_old)
                    psg = ps_g.tile([128, NT], F32)
                    emit_main(wg_sb, psg, x_t, k)
                    nc.scalar.activation(
                        out=g_cur[:, k, 0, 3:3 + NT], in_=psg[:],
                        func=AF.Copy, scale=SG8 / SPS)
                    nc.vector.tensor_scalar(
                        g_cur[:, k, 1, 2:2 + NT], psg[:], SG8 / SPS, None,
                        ALU.mult)
                    gq.append((k, psg))
                for kk, psg_old in gq:
                    emit_taps(g_cur, kk, psg_old, 0)
                    sgs[kk] = emit_silu(psg_old)
                upend = None
                for k in range(FC):
                    if upend is not None:
                        emit_taps(u_cur, k - 1, upend, 1)
                        emit_h(h_t, k - 1, upend, sgs[k - 1])
                    psu = ps_u.tile([128, NT], F32)
                    emit_main(wu_sb, psu, x_t, k)
                    nc.vector.tensor_scalar(
                        u_cur[:, k, 0, 3:3 + NT], psu[:], SG8 / SPS, None,
                        ALU.mult)
                    nc.scalar.activation(
                        out=u_cur[:, k, 1, 2:2 + NT], in_=psu[:],
                        func=AF.Copy, scale=SG8 / SPS)
                    upend = psu
                emit_taps(u_cur, FC - 1, upend, 1)
                emit_h(h_t, FC - 1, upend, sgs[FC - 1])
                g_prev, u_prev = g_cur, u_cur
                h_tiles[0] = h_t

            # pending (g_cur, u_cur, h_t, fc, psg, psu): projections whose
            # conv taps + silu + h are emitted one fc later, so the in-order
            # PE queue never waits on the evac->tap dependency.
            pend = [None]

            def tile_phase(tt):
                """Emit gate/up(tt) interleaved with down(tt-1)."""
                nonlocal g_prev, u_prev
                if tt < N_TILES:
                    x_t = x_tiles.pop(tt)
                    g_cur, u_cur, h_t = new_bufs(tt)
                for k in range(FC):
                    prev = pend[0]
                    pend[0] = None
                    if prev is not None:
                        # PE: taps for the previous unit first (evacs landed
                        # last iteration)
                        emit_taps(prev[0], prev[3], prev[4], 0)
                        emit_taps(prev[1], prev[3], prev[5], 1)
                    if tt < N_TILES:
                        psg = ps_g.tile([128, NT], F32)
                        psu = ps_u.tile([128, NT], F32)
                        emit_main(wg_sb, psg, x_t, k)
                        # fp8 copies (and their 1-token-shifted twins) of the
                        # pre-conv projections for the taps and next halo;
                        # split across ACT and DVE per branch
                        nc.scalar.activation(
                            out=g_cur[:, k, 0, 3:3 + NT], in_=psg[:],
                            func=AF.Copy, scale=SG8 / SPS)
                        nc.vector.tensor_scalar(
                            g_cur[:, k, 1, 2:2 + NT], psg[:], SG8 / SPS, None,
                            ALU.mult)
                        emit_main(wu_sb, psu, x_t, k)
                        nc.vector.tensor_scalar(
                            u_cur[:, k, 0, 3:3 + NT], psu[:], SG8 / SPS, None,
                            ALU.mult)
                        nc.scalar.activation(
                            out=u_cur[:, k, 1, 2:2 + NT], in_=psu[:],
                            func=AF.Copy, scale=SG8 / SPS)
                    if prev is not None:
                        # silu + h for the previous unit, after this unit's
                        # evacs in the ACT/DVE queues (shortens the
                        # PE-critical evac->tap chain)
                        sg = emit_silu(prev[4])
                        emit_h(prev[2], prev[3], prev[5], sg)
                    if tt < N_TILES:
                        pend[0] = (g_cur, u_cur, h_t, k, psg, psu)
                    if tt >= 1:
                        down_pair(tt - 1, k, last=(tt == N_TILES))
                if tt >= 1:
                    h_tiles.pop(tt - 1)
                if tt < N_TILES:
                    g_prev, u_prev = g_cur, u_cur
                    h_tiles[tt] = h_t

            tile0_phase()
            for tt in range(1, N_TILES + 1):
                if tt + 1 <= N_TILES - 1:
                    load_x(tt + 1)
                tile_phase(tt)

    nc.compile()
    return nc


_NC_CACHE = None


def _get_nc():
    global _NC_CACHE
    if _NC_CACHE is None:
        _NC_CACHE = build_nc()
    return _NC_CACHE


def _split8(a, scale):
    s = np.asarray(a * np.float32(scale), dtype=np.float32)
    hi = s.astype(E4NP)
    lo = (s - hi.astype(np.float32)).astype(E4NP)
    return hi, lo


def _prep_inputs(x, w_gate, w_up, w_down, conv_w):
    xh, xl = _split8(np.ascontiguousarray(x.reshape(TT, D).T), SX)  # [D, TT]
    # compact conv taps [p, fc, branch, k], scaled for the diag stationaries
    cwf = conv_w.reshape(2, NCORES, FC, 128, 4)                 # [br,c,fc,p,k]
    eye = np.eye(128, dtype=np.float16)
    in_maps = []
    for c in range(NCORES):
        fs = slice(c * FC_PER_CORE, (c + 1) * FC_PER_CORE)
        wgh, wgl = _split8(np.ascontiguousarray(w_gate[fs].T), SW)
        wuh, wul = _split8(np.ascontiguousarray(w_up[fs].T), SW)
        wdh, wdl = _split8(np.ascontiguousarray(w_down[:, fs].T), SWD)
        cwc = np.ascontiguousarray(
            cwf[:, c].transpose(2, 1, 0, 3)) * (SPS / SG8)      # [p,fc,br,k]
        in_maps.append({
            "xhT": xh, "xlT": xl, "wghT": wgh, "wglT": wgl,
            "wuhT": wuh, "wulT": wul, "wdhT": wdh, "wdlT": wdl,
            "cw": cwc.astype(np.float32), "eye": eye})
    return in_maps


def run_spmd(in_maps, **kwargs):
    nc = _get_nc()
    return run_bass_kernel_spmd(
        nc, in_maps, core_ids=list(range(NCORES)), **kwargs)


def kernel(x, w_gate, w_up, w_down, conv_w):
    in_maps = _prep_inputs(
        np.asarray(x, dtype=np.float32), np.asarray(w_gate, dtype=np.float32),
        np.asarray(w_up, dtype=np.float32),
        np.asarray(w_down, dtype=np.float32),
        np.asarray(conv_w, dtype=np.float32))
    res = run_spmd(in_maps)
    acc = np.zeros((D, TT), np.float32)
    for r in res.results:
        acc += r["outT"].astype(np.float32)
    return np.ascontiguousarray(acc.T).reshape(B, T, D)


# revision 4
# speedup vs baseline: 1.0106x; 1.0098x over previous
"""CanonGLU feedforward layer on 8 TRN2 NeuronCores — fp8 DoubleRow,
split-operand (hi+lo) version.

Math (per reference):
    gate = x @ w_gate.T ; up = x @ w_up.T            # [B,T,F]
    gate += causal_dconv(gate, conv_w[:F]) ; up += causal_dconv(up, conv_w[F:])
    out  = (up * silu(gate)) @ w_down.T              # [B,T,D]

Sharding: tensor-parallel over d_ff. Core c owns f-slice [c*1024,(c+1)*1024)
of w_gate/w_up/conv_w (column parallel) and w_down (row parallel); x
replicated.  Each core computes a full-shape partial output; the host sums
the 8 partials (the "all-reduce").

Precision scheme: e4m3 DoubleRow matmuls run at 0.5 cyc per output column
with K=256 per instruction — 4x the fp16 rate.  A single e4m3 quantization
of ANY matmul operand exceeds the 2e-2 gate (measured 2.6-3.2e-2 each), so
every operand is split v ~ hi + lo with hi = e4m3(v*S) and lo = e4m3(v*S -
hi) (same scale — no 16x mismatch in the psum), and each matmul becomes
three DoubleRow groups accumulating into one psum:
    x_hi@w_hi + x_hi@w_lo + x_lo@w_hi        (lo*lo term ~2^-8 rel, dropped)
That is 1.5 fp16-equivalents of PE time per matmul = 75% of the fp16
kernel's PE cost, at ~fp16 precision (measured end-to-end 1.08e-2).

The depthwise conv runs ON THE PE as diagonal-stationary DoubleRow matmuls
accumulating into the projection psum: tap k needs c_k[f]*gate[f, t+k-3] =
diag(c_k) @ shift_k(gate8), built from a single-e4m3 copy of the gate
(tap terms are ~10% magnitude; their quantization is in the 1.08e-2 total).

Scales (e4m3 range [2^-9, 240]):
    x*SX (SX=8), w_gate/w_up*SW (SW=512) -> proj psum scale SPS = 4096
    gate8 copy *SG8 (16) -> |gate|<=7.6 -> 122; diag c*SPS/SG8 = c*256 <= 116
    h*SH (SH=4, |h|<=35 -> 140), w_down*SWD (512) -> out psum scale 2048

Engine budget per core (64 (fc,tile) units, TimelineSim cost model):
    PE   ~527us  (mains 2x24 DR + taps 2x2 DR per unit; down 12 DR x 16dc)
    ACT  ~196us  (gate evac, silu, h_hi/h_lo casts, half the down evacs)
    DVE  ~165us  (up evac, h16, h residual, half the down evacs)
    Pool — GPSIMD cannot access PSUM (walrus birverifier); idle.
    DMA  ~130us  (in 29.4MB fp8 hi+lo, out 16.8MB fp16 partials)
"""

import numpy as np
import ml_dtypes

import concourse.bass as bass
import concourse.mybir as mybir
import concourse.tile as tile
from concourse import bacc
from concourse.bass_utils import run_bass_kernel_spmd

F16 = mybir.dt.float16
F32 = mybir.dt.float32
F8 = mybir.dt.float8e4
E4NP = ml_dtypes.float8_e4m3
AF = mybir.ActivationFunctionType
ALU = mybir.AluOpType
DR = mybir.MatmulPerfMode.DoubleRow

B, T, D, F = 2, 2048, 2048, 8192
NCORES = 8
FC_PER_CORE = F // NCORES          # 1024 f per core
TT = B * T                         # 4096 tokens total
NT = 512                           # token tile (one PSUM bank of fp32)
N_TILES = TT // NT                 # 8
TILES_PER_BATCH = T // NT          # 4 (conv halo resets at 0 and 4)
DC = D // 128                      # 16 d-chunks
FC = FC_PER_CORE // 128            # 8 f-chunks per core
GROW = NT + 4                      # conv fp8 row: 3 halo + 512 data + pad
# (pad keeps the slot stride EVEN — an odd-stride DoubleRow moving
# operand crashes the hardware fetch)

SX = 8.0                           # x fp8 scale
SW = 512.0                         # w_gate/w_up fp8 scale
SPS = SX * SW                      # gate/up psum scale (4096)
SG8 = 16.0                         # gate/up fp8 copy scale
SH = 4.0                           # h fp8 scale (|h| <= ~35)
SWD = 512.0                        # w_down fp8 scale
SOUT = SH * SWD                    # down psum scale (2048)


def build_nc():
    nc = bacc.Bacc(None, target_bir_lowering=False, debug=False)

    xhT = nc.dram_tensor("xhT", [D, TT], F8, kind="ExternalInput")
    xlT = nc.dram_tensor("xlT", [D, TT], F8, kind="ExternalInput")
    wghT = nc.dram_tensor("wghT", [D, FC_PER_CORE], F8, kind="ExternalInput")
    wglT = nc.dram_tensor("wglT", [D, FC_PER_CORE], F8, kind="ExternalInput")
    wuhT = nc.dram_tensor("wuhT", [D, FC_PER_CORE], F8, kind="ExternalInput")
    wulT = nc.dram_tensor("wulT", [D, FC_PER_CORE], F8, kind="ExternalInput")
    wdhT = nc.dram_tensor("wdhT", [FC_PER_CORE, D], F8, kind="ExternalInput")
    wdlT = nc.dram_tensor("wdlT", [FC_PER_CORE, D], F8, kind="ExternalInput")
    # compact conv taps [p, fc, branch, k] and a 128x128 identity mask; the
    # diagonal stationary tiles are built on-device (idle DVE at startup).
    cw = nc.dram_tensor("cw", [128, FC, 2, 4], F32, kind="ExternalInput")
    eye = nc.dram_tensor("eye", [128, 128], F16, kind="ExternalInput")
    outT = nc.dram_tensor("outT", [D, TT], F16, kind="ExternalOutput")

    def rearr(t, pat):
        return t.rearrange(pat, p=128)

    xhTr = rearr(xhT, "(dc p) t -> p dc t")
    xlTr = rearr(xlT, "(dc p) t -> p dc t")
    wgTr = [rearr(wghT, "(dc p) f -> p dc f"), rearr(wglT, "(dc p) f -> p dc f")]
    wuTr = [rearr(wuhT, "(dc p) f -> p dc f"), rearr(wulT, "(dc p) f -> p dc f")]
    wdTr = [rearr(wdhT, "(fc p) d -> p fc d"), rearr(wdlT, "(fc p) d -> p fc d")]
    outTr = rearr(outT, "(dp p) t -> p dp t")

    with tile.TileContext(nc) as tc:
        with (
            tc.tile_pool(name="consts", bufs=1) as consts,
            tc.tile_pool(name="xp", bufs=2) as xpool,
            tc.tile_pool(name="gb", bufs=2) as gbpool,
            tc.tile_pool(name="ub", bufs=2) as ubpool,
            tc.tile_pool(name="hp", bufs=2) as hpool,
            tc.tile_pool(name="sgp", bufs=9) as sgpool,
            tc.tile_pool(name="scr", bufs=2) as scrpool,
            tc.tile_pool(name="op", bufs=3) as opool,
            tc.tile_pool(name="psg", bufs=3, space="PSUM") as ps_g,
            tc.tile_pool(name="psu", bufs=2, space="PSUM") as ps_u,
            tc.tile_pool(name="pso", bufs=3, space="PSUM") as ps_o,
        ):
            wg_sb = consts.tile([128, 2, DC, FC_PER_CORE], F8)
            wu_sb = consts.tile([128, 2, DC, FC_PER_CORE], F8)
            wd_sb = consts.tile([128, 2, FC, D], F8)
            cd_sb = consts.tile([128, FC, 2, 2, 2, 128], F8)
            cw_sb = consts.tile([128, FC, 2, 4], F32)
            eye_sb = consts.tile([128, 128], F16)

            x_tiles = {}

            def load_x(tt):
                # one DMA per (tile, half): HWDGE descriptor-gen holds ~627ns
                # per dma_start, so fine-grained loads serialize on it
                x_t = xpool.tile([128, 2, DC, NT], F8)
                nc.sync.dma_start(out=x_t[:, 0], in_=xhTr[:, :, bass.ts(tt, NT)])
                nc.sync.dma_start(out=x_t[:, 1], in_=xlTr[:, :, bass.ts(tt, NT)])
                x_tiles[tt] = x_t

            # PE warmup: dummy matmuls on zeroed SBUF fill the startup DMA
            # wait (w_gate hi+lo 4.2MB + x0 2MB ~ 17us) so the PE p-state
            # reaches (and keeps) 2.4 GHz before the first real matmul.
            warm_sb = consts.tile([128, NT], F16)
            nc.gpsimd.memset(warm_sb[:], 0.0)
            warm_ps = ps_g.tile([128, NT], F32, name="psg")
            for _ in range(20):
                nc.tensor.matmul(
                    warm_ps[:], warm_sb[:, 0:128], warm_sb[:],
                    start=True, stop=True)

            # startup DMA order = first-need order for the tile-0 gate-first
            # schedule.
            nc.sync.dma_start(out=eye_sb[:], in_=eye[:])
            nc.sync.dma_start(out=cw_sb[:], in_=cw[:])
            nc.sync.dma_start(out=wg_sb[:, 0], in_=wgTr[0][:])
            load_x(0)
            nc.sync.dma_start(out=wg_sb[:, 1], in_=wgTr[1][:])
            nc.sync.dma_start(out=wu_sb[:, 0], in_=wuTr[0][:])
            nc.sync.dma_start(out=wu_sb[:, 1], in_=wuTr[1][:])
            load_x(1)
            nc.sync.dma_start(out=wd_sb[:, 0], in_=wdTr[0][:])
            nc.sync.dma_start(out=wd_sb[:, 1], in_=wdTr[1][:])
            # build the diagonal tap stationaries on the (startup-idle) DVE:
            # cd[:, fc, br, pr, i, :] = eye * c_{2pr+i}[partition]
            for fc in range(FC):
                for br in range(2):
                    for k in range(4):
                        nc.vector.tensor_scalar(
                            cd_sb[:, fc, br, k // 2, k % 2, :], eye_sb[:],
                            cw_sb[:, fc, br, k:k + 1], None, ALU.mult)

            g_prev = u_prev = None
            h_tiles = {}

            def emit_main(w_sb, psp, x_t, k,
                          groups=((0, 0), (1, 0), (0, 1)), start=True):
                # three DoubleRow groups accumulate into one psum; the conv
                # taps close the group later.
                first = start
                for wi, xi in groups:
                    for i in range(DC // 2):
                        nc.tensor.matmul(
                            psp[:],
                            w_sb[:, wi, 2 * i:2 * i + 2, bass.ts(k, 128)],
                            x_t[:, xi, 2 * i:2 * i + 2, :],
                            start=first, stop=False, perf_mode=DR,
                            skip_group_check=not first)
                        first = False

            def emit_taps(buf, k, psp, br):
                # conv taps on the PE: diag(c_k)@shift_k accumulated into the
                # projection psum (residual term is already there).  buf slot
                # 0 holds gate8, slot 1 the 1-token-shifted copy, so the
                # DoubleRow pair (shift 2pr, 2pr+1) is the non-overlapping
                # strided view [p, 2, NT] at offset 2pr — an overlapping AP
                # on the moving operand kills the hardware fetch.
                for pr in range(2):
                    nc.tensor.matmul(
                        psp[:],
                        cd_sb[:, k, br, pr, :, :],
                        buf[:, k, :, 2 * pr:2 * pr + NT],
                        start=False, stop=(pr == 1),
                        perf_mode=DR, skip_group_check=True)

            def emit_silu(psg):
                sg = sgpool.tile([128, NT], F16)
                nc.scalar.activation(
                    out=sg[:], in_=psg[:], func=AF.Silu, scale=1.0 / SPS)
                return sg

            def emit_h(h_t, k, psu, sg):
                # h*SH = (psum_u * SH/SPS) * sg, split hi (fp8) + lo (fp8 of
                # the fp16 residual) for the split down-proj
                h16 = scrpool.tile([128, NT], F16, name="h16")
                nc.vector.scalar_tensor_tensor(
                    out=h16[:], in0=psu[:], scalar=SH / SPS,
                    in1=sg[:], op0=ALU.mult, op1=ALU.mult)
                nc.scalar.activation(
                    out=h_t[:, 0, k, :], in_=h16[:], func=AF.Copy)
                r16 = scrpool.tile([128, NT], F16, name="r16")
                nc.vector.tensor_tensor(
                    out=r16[:], in0=h16[:], in1=h_t[:, 0, k, :], op=ALU.subtract)
                nc.scalar.activation(
                    out=h_t[:, 1, k, :], in_=r16[:], func=AF.Copy)

            def down_pair(tt, k, last=False):
                """Down-proj for dc = 2k, 2k+1: three DoubleRow groups per
                psum (hh, hl, lh), evacs alternated ACT/DVE (GPSIMD cannot
                read PSUM), one out-DMA per pair."""
                h_t = h_tiles[tt]
                o_sb = opool.tile([128, 2, NT], F16)
                for half in range(2):
                    dc = 2 * k + half
                    pool, tag = ((ps_o, "pso"), (ps_g, "psg"),
                                 (ps_u, "psu"))[dc % 3 if last else 0]
                    pso = pool.tile([128, NT], F32, name=tag)
                    first = True
                    for wi, hi in ((0, 0), (1, 0), (0, 1)):
                        for j in range(FC // 2):
                            nc.tensor.matmul(
                                pso[:],
                                wd_sb[:, wi, 2 * j:2 * j + 2, bass.ts(dc, 128)],
                                h_t[:, hi, 2 * j:2 * j + 2, :],
                                start=first,
                                stop=(wi == 0 and hi == 1 and j == FC // 2 - 1),
                                perf_mode=DR, skip_group_check=not first)
                            first = False
                    o_slot = o_sb[:, half, :]
                    if dc % 2 == 0:
                        nc.scalar.activation(
                            out=o_slot, in_=pso[:], func=AF.Copy,
                            scale=1.0 / SOUT)
                    else:
                        nc.vector.tensor_scalar(
                            o_slot, pso[:], 1.0 / SOUT, None, ALU.mult)
                if last:
                    # tail drain: per-dc DMAs so the final transfer only
                    # waits its own evac
                    for half in range(2):
                        eng = nc.sync if half == 0 else nc.scalar
                        eng.dma_start(
                            out=outTr[:, 2 * k + half, bass.ts(tt, NT)],
                            in_=o_sb[:, half, :])
                else:
                    eng = nc.sync if k % 2 == 0 else nc.scalar
                    eng.dma_start(
                        out=outTr[:, 2 * k:2 * k + 2, bass.ts(tt, NT)],
                        in_=o_sb[:])

            def new_bufs(tt):
                # slot 0: gate8/up8 at token offset -3 (3 halo + 512 data);
                # slot 1: the same data shifted one token (2 halo + 512 data)
                g_cur = gbpool.tile([128, FC, 2, GROW], F8)
                u_cur = ubpool.tile([128, FC, 2, GROW], F8)
                h_t = hpool.tile([128, 2, FC, NT], F8)
                # conv halo: last tokens of the previous tile's fp8 copies
                # (zeros at the start of each batch — causal left pad).
                for br, (buf, prev) in enumerate(((g_cur, g_prev),
                                                  (u_cur, u_prev))):
                    if tt % TILES_PER_BATCH == 0:
                        nc.vector.memset(buf[:, :, 0, 0:3], 0.0)
                        nc.vector.memset(buf[:, :, 1, 0:2], 0.0)
                    else:
                        nc.vector.tensor_copy(
                            out=buf[:, :, 0, 0:3],
                            in_=prev[:, :, 0, NT:NT + 3])
                        nc.vector.tensor_copy(
                            out=buf[:, :, 1, 0:2],
                            in_=prev[:, :, 1, NT:NT + 2])
                return g_cur, u_cur, h_t

            def tile0_phase():
                """Tile 0 streams gate first (only w_gate + x(0) must have
                landed), then up — hides the w_up DMA behind the gate pass
                instead of stalling the PE."""
                nonlocal g_prev, u_prev
                x_t = x_tiles.pop(0)
                g_cur, u_cur, h_t = new_bufs(0)
                sgs = {}
                # gate taps lag TWO mains so the first tap (which needs the
                # x_lo DMA-built diags) never stalls the in-order PE queue
                # startup waves: while w_gate_lo is in flight, run the
                # groups needing only w_gate_hi + x(0) for the first three
                # fc units (all three psg banks)
                early = {}
                for k in range(3):
                    psg = ps_g.tile([128, NT], F32, name="psg")
                    emit_main(wg_sb, psg, x_t, k, groups=((0, 0),))
                    early[k] = psg
                for k in range(3):
                    emit_main(wg_sb, early[k], x_t, k, groups=((0, 1),),
                              start=False)
                gq = []
                for k in range(FC):
                    if len(gq) == 2:
                        kk, psg_old = gq.pop(0)
                        emit_taps(g_cur, kk, psg_old, 0)
                        sgs[kk] = emit_silu(psg_old)
                    if k in early:
                        psg = early.pop(k)
                        emit_main(wg_sb, psg, x_t, k, groups=((1, 0),),
                                  start=False)
                    else:
                        psg = ps_g.tile([128, NT], F32, name="psg")
                        emit_main(wg_sb, psg, x_t, k)
                    nc.scalar.activation(
                        out=g_cur[:, k, 0, 3:3 + NT], in_=psg[:],
                        func=AF.Copy, scale=SG8 / SPS)
                    nc.vector.tensor_scalar(
                        g_cur[:, k, 1, 2:2 + NT], psg[:], SG8 / SPS, None,
                        ALU.mult)
                    gq.append((k, psg))
                for kk, psg_old in gq:
                    emit_taps(g_cur, kk, psg_old, 0)
                    sgs[kk] = emit_silu(psg_old)
                upend = None
                for k in range(FC):
                    if upend is not None:
                        emit_taps(u_cur, k - 1, upend, 1)
                        emit_h(h_t, k - 1, upend, sgs[k - 1])
                    psu = ps_u.tile([128, NT], F32)
                    emit_main(wu_sb, psu, x_t, k)
                    nc.vector.tensor_scalar(
                        u_cur[:, k, 0, 3:3 + NT], psu[:], SG8 / SPS, None,
                        ALU.mult)
                    nc.scalar.activation(
                        out=u_cur[:, k, 1, 2:2 + NT], in_=psu[:],
                        func=AF.Copy, scale=SG8 / SPS)
                    upend = psu
                emit_taps(u_cur, FC - 1, upend, 1)
                emit_h(h_t, FC - 1, upend, sgs[FC - 1])
                g_prev, u_prev = g_cur, u_cur
                h_tiles[0] = h_t

            # pending (g_cur, u_cur, h_t, fc, psg, psu): projections whose
            # conv taps + silu + h are emitted one fc later, so the in-order
            # PE queue never waits on the evac->tap dependency.
            pend = [None]

            def tile_phase(tt):
                """Emit gate/up(tt) interleaved with down(tt-1)."""
                nonlocal g_prev, u_prev
                if tt < N_TILES:
                    x_t = x_tiles.pop(tt)
                    g_cur, u_cur, h_t = new_bufs(tt)
                for k in range(FC):
                    prev = pend[0]
                    pend[0] = None
                    if prev is not None:
                        # PE: taps for the previous unit first (evacs landed
                        # last iteration)
                        emit_taps(prev[0], prev[3], prev[4], 0)
                        emit_taps(prev[1], prev[3], prev[5], 1)
                    if tt < N_TILES:
                        psg = ps_g.tile([128, NT], F32)
                        psu = ps_u.tile([128, NT], F32)
                        emit_main(wg_sb, psg, x_t, k)
                        # fp8 copies (and their 1-token-shifted twins) of the
                        # pre-conv projections for the taps and next halo;
                        # split across ACT and DVE per branch
                        nc.scalar.activation(
                            out=g_cur[:, k, 0, 3:3 + NT], in_=psg[:],
                            func=AF.Copy, scale=SG8 / SPS)
                        nc.vector.tensor_scalar(
                            g_cur[:, k, 1, 2:2 + NT], psg[:], SG8 / SPS, None,
                            ALU.mult)
                        emit_main(wu_sb, psu, x_t, k)
                        nc.vector.tensor_scalar(
                            u_cur[:, k, 0, 3:3 + NT], psu[:], SG8 / SPS, None,
                            ALU.mult)
                        nc.scalar.activation(
                            out=u_cur[:, k, 1, 2:2 + NT], in_=psu[:],
                            func=AF.Copy, scale=SG8 / SPS)
                    if prev is not None:
                        # silu + h for the previous unit, after this unit's
                        # evacs in the ACT/DVE queues (shortens the
                        # PE-critical evac->tap chain)
                        sg = emit_silu(prev[4])
                        emit_h(prev[2], prev[3], prev[5], sg)
                    if tt < N_TILES:
                        pend[0] = (g_cur, u_cur, h_t, k, psg, psu)
                    if tt >= 1:
                        down_pair(tt - 1, k, last=(tt == N_TILES))
                if tt >= 1:
                    h_tiles.pop(tt - 1)
                if tt < N_TILES:
                    g_prev, u_prev = g_cur, u_cur
                    h_tiles[tt] = h_t

            tile0_phase()
            for tt in range(1, N_TILES + 1):
                if tt + 1 <= N_TILES - 1:
                    load_x(tt + 1)
                tile_phase(tt)

    nc.compile()
    return nc


_NC_CACHE = None


def _get_nc():
    global _NC_CACHE
    if _NC_CACHE is None:
        _NC_CACHE = build_nc()
    return _NC_CACHE


def _split8(a, scale):
    s = np.asarray(a * np.float32(scale), dtype=np.float32)
    hi = s.astype(E4NP)
    lo = (s - hi.astype(np.float32)).astype(E4NP)
    return hi, lo


def _prep_inputs(x, w_gate, w_up, w_down, conv_w):
    xh, xl = _split8(np.ascontiguousarray(x.reshape(TT, D).T), SX)  # [D, TT]
    # compact conv taps [p, fc, branch, k], scaled for the diag stationaries
    cwf = conv_w.reshape(2, NCORES, FC, 128, 4)                 # [br,c,fc,p,k]
    eye = np.eye(128, dtype=np.float16)
    in_maps = []
    for c in range(NCORES):
        fs = slice(c * FC_PER_CORE, (c + 1) * FC_PER_CORE)
        wgh, wgl = _split8(np.ascontiguousarray(w_gate[fs].T), SW)
        wuh, wul = _split8(np.ascontiguousarray(w_up[fs].T), SW)
        wdh, wdl = _split8(np.ascontiguousarray(w_down[:, fs].T), SWD)
        cwc = np.ascontiguousarray(
            cwf[:, c].transpose(2, 1, 0, 3)) * (SPS / SG8)      # [p,fc,br,k]
        in_maps.append({
            "xhT": xh, "xlT": xl, "wghT": wgh, "wglT": wgl,
            "wuhT": wuh, "wulT": wul, "wdhT": wdh, "wdlT": wdl,
            "cw": cwc.astype(np.float32), "eye": eye})
    return in_maps


def run_spmd(in_maps, **kwargs):
    nc = _get_nc()
    return run_bass_kernel_spmd(
        nc, in_maps, core_ids=list(range(NCORES)), **kwargs)


def kernel(x, w_gate, w_up, w_down, conv_w):
    in_maps = _prep_inputs(
        np.asarray(x, dtype=np.float32), np.asarray(w_gate, dtype=np.float32),
        np.asarray(w_up, dtype=np.float32),
        np.asarray(w_down, dtype=np.float32),
        np.asarray(conv_w, dtype=np.float32))
    res = run_spmd(in_maps)
    acc = np.zeros((D, TT), np.float32)
    for r in res.results:
        acc += r["outT"].astype(np.float32)
    return np.ascontiguousarray(acc.T).reshape(B, T, D)


# revision 5
# speedup vs baseline: 1.0204x; 1.0097x over previous
"""CanonGLU feedforward layer on 8 TRN2 NeuronCores — fp8 DoubleRow,
split-operand (hi+lo) version.

Math (per reference):
    gate = x @ w_gate.T ; up = x @ w_up.T            # [B,T,F]
    gate += causal_dconv(gate, conv_w[:F]) ; up += causal_dconv(up, conv_w[F:])
    out  = (up * silu(gate)) @ w_down.T              # [B,T,D]

Sharding: tensor-parallel over d_ff. Core c owns f-slice [c*1024,(c+1)*1024)
of w_gate/w_up/conv_w (column parallel) and w_down (row parallel); x
replicated.  Each core computes a full-shape partial output; the host sums
the 8 partials (the "all-reduce").

Precision scheme: e4m3 DoubleRow matmuls run at 0.5 cyc per output column
with K=256 per instruction — 4x the fp16 rate.  A single e4m3 quantization
of ANY matmul operand exceeds the 2e-2 gate (measured 2.6-3.2e-2 each), so
every operand is split v ~ hi + lo with hi = e4m3(v*S) and lo = e4m3(v*S -
hi) (same scale — no 16x mismatch in the psum), and each matmul becomes
three DoubleRow groups accumulating into one psum:
    x_hi@w_hi + x_hi@w_lo + x_lo@w_hi        (lo*lo term ~2^-8 rel, dropped)
That is 1.5 fp16-equivalents of PE time per matmul = 75% of the fp16
kernel's PE cost, at ~fp16 precision (measured end-to-end 1.08e-2).

The depthwise conv runs ON THE PE as diagonal-stationary DoubleRow matmuls
accumulating into the projection psum: tap k needs c_k[f]*gate[f, t+k-3] =
diag(c_k) @ shift_k(gate8), built from a single-e4m3 copy of the gate
(tap terms are ~10% magnitude; their quantization is in the 1.08e-2 total).

Scales (e4m3 range [2^-9, 240]):
    x*SX (SX=8), w_gate/w_up*SW (SW=512) -> proj psum scale SPS = 4096
    gate8 copy *SG8 (16) -> |gate|<=7.6 -> 122; diag c*SPS/SG8 = c*256 <= 116
    h*SH (SH=4, |h|<=35 -> 140), w_down*SWD (512) -> out psum scale 2048

Engine budget per core (64 (fc,tile) units, TimelineSim cost model):
    PE   ~527us  (mains 2x24 DR + taps 2x2 DR per unit; down 12 DR x 16dc)
    ACT  ~196us  (gate evac, silu, h_hi/h_lo casts, half the down evacs)
    DVE  ~165us  (up evac, h16, h residual, half the down evacs)
    Pool — GPSIMD cannot access PSUM (walrus birverifier); idle.
    DMA  ~130us  (in 29.4MB fp8 hi+lo, out 16.8MB fp16 partials)
"""

import numpy as np
import ml_dtypes

import concourse.bass as bass
import concourse.mybir as mybir
import concourse.tile as tile
from concourse import bacc
from concourse.bass_utils import run_bass_kernel_spmd

F16 = mybir.dt.float16
F32 = mybir.dt.float32
F8 = mybir.dt.float8e4
E4NP = ml_dtypes.float8_e4m3
AF = mybir.ActivationFunctionType
ALU = mybir.AluOpType
DR = mybir.MatmulPerfMode.DoubleRow

B, T, D, F = 2, 2048, 2048, 8192
NCORES = 8
FC_PER_CORE = F // NCORES          # 1024 f per core
TT = B * T                         # 4096 tokens total
NT = 512                           # token tile (one PSUM bank of fp32)
N_TILES = TT // NT                 # 8
TILES_PER_BATCH = T // NT          # 4 (conv halo resets at 0 and 4)
DC = D // 128                      # 16 d-chunks
FC = FC_PER_CORE // 128            # 8 f-chunks per core
GROW = NT + 4                      # conv fp8 row: 3 halo + 512 data + pad
# (pad keeps the slot stride EVEN — an odd-stride DoubleRow moving
# operand crashes the hardware fetch)

SX = 8.0                           # x fp8 scale
SW = 512.0                         # w_gate/w_up fp8 scale
SPS = SX * SW                      # gate/up psum scale (4096)
SG8 = 16.0                         # gate/up fp8 copy scale
SH = 4.0                           # h fp8 scale (|h| <= ~35)
SWD = 512.0                        # w_down fp8 scale
SOUT = SH * SWD                    # down psum scale (2048)


def build_nc():
    nc = bacc.Bacc(None, target_bir_lowering=False, debug=False)

    xhT = nc.dram_tensor("xhT", [D, TT], F8, kind="ExternalInput")
    xlT = nc.dram_tensor("xlT", [D, TT], F8, kind="ExternalInput")
    wghT = nc.dram_tensor("wghT", [D, FC_PER_CORE], F8, kind="ExternalInput")
    wglT = nc.dram_tensor("wglT", [D, FC_PER_CORE], F8, kind="ExternalInput")
    wuhT = nc.dram_tensor("wuhT", [D, FC_PER_CORE], F8, kind="ExternalInput")
    wulT = nc.dram_tensor("wulT", [D, FC_PER_CORE], F8, kind="ExternalInput")
    wdhT = nc.dram_tensor("wdhT", [FC_PER_CORE, D], F8, kind="ExternalInput")
    wdlT = nc.dram_tensor("wdlT", [FC_PER_CORE, D], F8, kind="ExternalInput")
    # compact conv taps [p, fc, branch, k] and a 128x128 identity mask; the
    # diagonal stationary tiles are built on-device (idle DVE at startup).
    cw = nc.dram_tensor("cw", [128, FC, 2, 4], F32, kind="ExternalInput")
    eye = nc.dram_tensor("eye", [128, 128], F16, kind="ExternalInput")
    outT = nc.dram_tensor("outT", [D, TT], F16, kind="ExternalOutput")

    def rearr(t, pat):
        return t.rearrange(pat, p=128)

    xhTr = rearr(xhT, "(dc p) t -> p dc t")
    xlTr = rearr(xlT, "(dc p) t -> p dc t")
    wgTr = [rearr(wghT, "(dc p) f -> p dc f"), rearr(wglT, "(dc p) f -> p dc f")]
    wuTr = [rearr(wuhT, "(dc p) f -> p dc f"), rearr(wulT, "(dc p) f -> p dc f")]
    wdTr = [rearr(wdhT, "(fc p) d -> p fc d"), rearr(wdlT, "(fc p) d -> p fc d")]
    outTr = rearr(outT, "(dp p) t -> p dp t")

    with tile.TileContext(nc) as tc:
        with (
            tc.tile_pool(name="consts", bufs=1) as consts,
            tc.tile_pool(name="xp", bufs=2) as xpool,
            tc.tile_pool(name="gb", bufs=2) as gbpool,
            tc.tile_pool(name="ub", bufs=2) as ubpool,
            tc.tile_pool(name="hp", bufs=2) as hpool,
            tc.tile_pool(name="sgp", bufs=9) as sgpool,
            tc.tile_pool(name="scr", bufs=2) as scrpool,
            tc.tile_pool(name="op", bufs=3) as opool,
            tc.tile_pool(name="psg", bufs=3, space="PSUM") as ps_g,
            tc.tile_pool(name="psu", bufs=2, space="PSUM") as ps_u,
            tc.tile_pool(name="pso", bufs=3, space="PSUM") as ps_o,
        ):
            wg_sb = consts.tile([128, 2, DC, FC_PER_CORE], F8)
            wu_sb = consts.tile([128, 2, DC, FC_PER_CORE], F8)
            wd_sb = consts.tile([128, 2, FC, D], F8)
            cd_sb = consts.tile([128, FC, 2, 2, 2, 128], F8)
            cw_sb = consts.tile([128, FC, 2, 4], F32)
            eye_sb = consts.tile([128, 128], F16)

            x_tiles = {}

            def load_x(tt):
                # one DMA per (tile, half): HWDGE descriptor-gen holds ~627ns
                # per dma_start, so fine-grained loads serialize on it
                x_t = xpool.tile([128, 2, DC, NT], F8)
                nc.sync.dma_start(out=x_t[:, 0], in_=xhTr[:, :, bass.ts(tt, NT)])
                nc.sync.dma_start(out=x_t[:, 1], in_=xlTr[:, :, bass.ts(tt, NT)])
                x_tiles[tt] = x_t

            # PE warmup: dummy matmuls on zeroed SBUF fill the startup DMA
            # wait (w_gate hi+lo 4.2MB + x0 2MB ~ 17us) so the PE p-state
            # reaches (and keeps) 2.4 GHz before the first real matmul.
            warm_sb = consts.tile([128, NT], F16)
            nc.gpsimd.memset(warm_sb[:], 0.0)
            warm_ps = ps_g.tile([128, NT], F32, name="psg")
            for _ in range(20):
                nc.tensor.matmul(
                    warm_ps[:], warm_sb[:, 0:128], warm_sb[:],
                    start=True, stop=True)

            # startup DMA order = first-need order for the tile-0 gate-first
            # schedule.
            nc.sync.dma_start(out=eye_sb[:], in_=eye[:])
            nc.sync.dma_start(out=cw_sb[:], in_=cw[:])
            nc.sync.dma_start(out=wg_sb[:, 0], in_=wgTr[0][:])
            load_x(0)
            nc.sync.dma_start(out=wg_sb[:, 1], in_=wgTr[1][:])
            nc.sync.dma_start(out=wu_sb[:, 0], in_=wuTr[0][:])
            nc.sync.dma_start(out=wu_sb[:, 1], in_=wuTr[1][:])
            load_x(1)
            nc.sync.dma_start(out=wd_sb[:, 0], in_=wdTr[0][:])
            nc.sync.dma_start(out=wd_sb[:, 1], in_=wdTr[1][:])
            # build the diagonal tap stationaries on the (startup-idle) DVE:
            # cd[:, fc, br, pr, i, :] = eye * c_{2pr+i}[partition]
            for fc in range(FC):
                for br in range(2):
                    for k in range(4):
                        nc.vector.tensor_scalar(
                            cd_sb[:, fc, br, k // 2, k % 2, :], eye_sb[:],
                            cw_sb[:, fc, br, k:k + 1], None, ALU.mult)

            g_prev = u_prev = None
            h_tiles = {}

            def emit_main(w_sb, psp, x_t, k,
                          groups=((0, 0), (1, 0), (0, 1)), start=True):
                # three DoubleRow groups accumulate into one psum; the conv
                # taps close the group later.
                first = start
                for wi, xi in groups:
                    for i in range(DC // 2):
                        nc.tensor.matmul(
                            psp[:],
                            w_sb[:, wi, 2 * i:2 * i + 2, bass.ts(k, 128)],
                            x_t[:, xi, 2 * i:2 * i + 2, :],
                            start=first, stop=False, perf_mode=DR,
                            skip_group_check=not first)
                        first = False

            def emit_taps(buf, k, psp, br):
                # conv taps on the PE: diag(c_k)@shift_k accumulated into the
                # projection psum (residual term is already there).  buf slot
                # 0 holds gate8, slot 1 the 1-token-shifted copy, so the
                # DoubleRow pair (shift 2pr, 2pr+1) is the non-overlapping
                # strided view [p, 2, NT] at offset 2pr — an overlapping AP
                # on the moving operand kills the hardware fetch.
                for pr in range(2):
                    nc.tensor.matmul(
                        psp[:],
                        cd_sb[:, k, br, pr, :, :],
                        buf[:, k, :, 2 * pr:2 * pr + NT],
                        start=False, stop=(pr == 1),
                        perf_mode=DR, skip_group_check=True)

            def emit_silu(psg):
                sg = sgpool.tile([128, NT], F16)
                nc.scalar.activation(
                    out=sg[:], in_=psg[:], func=AF.Silu, scale=1.0 / SPS)
                return sg

            def emit_h(h_t, k, psu, sg):
                # h*SH = (psum_u * SH/SPS) * sg, split hi (fp8) + lo (fp8 of
                # the fp16 residual) for the split down-proj
                h16 = scrpool.tile([128, NT], F16, name="h16")
                nc.vector.scalar_tensor_tensor(
                    out=h16[:], in0=psu[:], scalar=SH / SPS,
                    in1=sg[:], op0=ALU.mult, op1=ALU.mult)
                nc.scalar.activation(
                    out=h_t[:, 0, k, :], in_=h16[:], func=AF.Copy)
                r16 = scrpool.tile([128, NT], F16, name="r16")
                nc.vector.tensor_tensor(
                    out=r16[:], in0=h16[:], in1=h_t[:, 0, k, :], op=ALU.subtract)
                nc.scalar.activation(
                    out=h_t[:, 1, k, :], in_=r16[:], func=AF.Copy)

            def down_pair(tt, k, last=False):
                """Down-proj for dc = 2k, 2k+1: three DoubleRow groups per
                psum (hh, hl, lh), evacs alternated ACT/DVE (GPSIMD cannot
                read PSUM), one out-DMA per pair."""
                h_t = h_tiles[tt]
                o_sb = opool.tile([128, 2, NT], F16)
                for half in range(2):
                    dc = 2 * k + half
                    pool, tag = ((ps_o, "pso"), (ps_g, "psg"),
                                 (ps_u, "psu"))[dc % 3 if last else 0]
                    pso = pool.tile([128, NT], F32, name=tag)
                    first = True
                    for wi, hi in ((0, 0), (1, 0), (0, 1)):
                        for j in range(FC // 2):
                            nc.tensor.matmul(
                                pso[:],
                                wd_sb[:, wi, 2 * j:2 * j + 2, bass.ts(dc, 128)],
                                h_t[:, hi, 2 * j:2 * j + 2, :],
                                start=first,
                                stop=(wi == 0 and hi == 1 and j == FC // 2 - 1),
                                perf_mode=DR, skip_group_check=not first)
                            first = False
                    o_slot = o_sb[:, half, :]
                    if dc % 2 == 0:
                        nc.scalar.activation(
                            out=o_slot, in_=pso[:], func=AF.Copy,
                            scale=1.0 / SOUT)
                    else:
                        nc.vector.tensor_scalar(
                            o_slot, pso[:], 1.0 / SOUT, None, ALU.mult)
                if last:
                    # tail drain: per-dc DMAs so the final transfer only
                    # waits its own evac
                    for half in range(2):
                        eng = nc.sync if half == 0 else nc.scalar
                        eng.dma_start(
                            out=outTr[:, 2 * k + half, bass.ts(tt, NT)],
                            in_=o_sb[:, half, :])
                else:
                    eng = nc.sync if k % 2 == 0 else nc.scalar
                    eng.dma_start(
                        out=outTr[:, 2 * k:2 * k + 2, bass.ts(tt, NT)],
                        in_=o_sb[:])

            def new_bufs(tt):
                # slot 0: gate8/up8 at token offset -3 (3 halo + 512 data);
                # slot 1: the same data shifted one token (2 halo + 512 data)
                g_cur = gbpool.tile([128, FC, 2, GROW], F8)
                u_cur = ubpool.tile([128, FC, 2, GROW], F8)
                h_t = hpool.tile([128, 2, FC, NT], F8)
                # conv halo: last tokens of the previous tile's fp8 copies
                # (zeros at the start of each batch — causal left pad).
                for br, (buf, prev) in enumerate(((g_cur, g_prev),
                                                  (u_cur, u_prev))):
                    if tt % TILES_PER_BATCH == 0:
                        nc.vector.memset(buf[:, :, 0, 0:3], 0.0)
                        nc.vector.memset(buf[:, :, 1, 0:2], 0.0)
                    else:
                        nc.vector.tensor_copy(
                            out=buf[:, :, 0, 0:3],
                            in_=prev[:, :, 0, NT:NT + 3])
                        nc.vector.tensor_copy(
                            out=buf[:, :, 1, 0:2],
                            in_=prev[:, :, 1, NT:NT + 2])
                return g_cur, u_cur, h_t

            def tile0_phase():
                """Tile 0 streams gate first (only w_gate + x(0) must have
                landed), then up — hides the w_up DMA behind the gate pass
                instead of stalling the PE."""
                nonlocal g_prev, u_prev
                x_t = x_tiles.pop(0)
                g_cur, u_cur, h_t = new_bufs(0)
                sgs = {}
                # gate taps lag TWO mains so the first tap (which needs the
                # x_lo DMA-built diags) never stalls the in-order PE queue
                # startup waves: while w_gate_lo is in flight, run the
                # groups needing only w_gate_hi + x(0) for the first three
                # fc units (all three psg banks)
                early = {}
                epool = [(ps_g, "psg")] * 3 + [(ps_u, "psu")] * 2 +                     [(ps_o, "pso")]
                for k in range(6):
                    pool, tag = epool[k]
                    psg = pool.tile([128, NT], F32, name=tag)
                    emit_main(wg_sb, psg, x_t, k, groups=((0, 0),))
                    early[k] = psg
                for k in range(6):
                    emit_main(wg_sb, early[k], x_t, k, groups=((0, 1),),
                              start=False)
                gq = []
                for k in range(FC):
                    if len(gq) == 2:
                        kk, psg_old = gq.pop(0)
                        emit_taps(g_cur, kk, psg_old, 0)
                        sgs[kk] = emit_silu(psg_old)
                    if k in early:
                        psg = early.pop(k)
                        emit_main(wg_sb, psg, x_t, k, groups=((1, 0),),
                                  start=False)
                    else:
                        psg = ps_g.tile([128, NT], F32, name="psg")
                        emit_main(wg_sb, psg, x_t, k)
                    nc.scalar.activation(
                        out=g_cur[:, k, 0, 3:3 + NT], in_=psg[:],
                        func=AF.Copy, scale=SG8 / SPS)
                    nc.vector.tensor_scalar(
                        g_cur[:, k, 1, 2:2 + NT], psg[:], SG8 / SPS, None,
                        ALU.mult)
                    gq.append((k, psg))
                for kk, psg_old in gq:
                    emit_taps(g_cur, kk, psg_old, 0)
                    sgs[kk] = emit_silu(psg_old)
                upend = None
                for k in range(FC):
                    if upend is not None:
                        emit_taps(u_cur, k - 1, upend, 1)
                        emit_h(h_t, k - 1, upend, sgs[k - 1])
                    psu = ps_u.tile([128, NT], F32)
                    emit_main(wu_sb, psu, x_t, k)
                    nc.vector.tensor_scalar(
                        u_cur[:, k, 0, 3:3 + NT], psu[:], SG8 / SPS, None,
                        ALU.mult)
                    nc.scalar.activation(
                        out=u_cur[:, k, 1, 2:2 + NT], in_=psu[:],
                        func=AF.Copy, scale=SG8 / SPS)
                    upend = psu
                emit_taps(u_cur, FC - 1, upend, 1)
                emit_h(h_t, FC - 1, upend, sgs[FC - 1])
                g_prev, u_prev = g_cur, u_cur
                h_tiles[0] = h_t

            # pending (g_cur, u_cur, h_t, fc, psg, psu): projections whose
            # conv taps + silu + h are emitted one fc later, so the in-order
            # PE queue never waits on the evac->tap dependency.
            pend = [None]

            def tile_phase(tt):
                """Emit gate/up(tt) interleaved with down(tt-1)."""
                nonlocal g_prev, u_prev
                if tt < N_TILES:
                    x_t = x_tiles.pop(tt)
                    g_cur, u_cur, h_t = new_bufs(tt)
                for k in range(FC):
                    prev = pend[0]
                    pend[0] = None
                    if prev is not None:
                        # PE: taps for the previous unit first (evacs landed
                        # last iteration)
                        emit_taps(prev[0], prev[3], prev[4], 0)
                        emit_taps(prev[1], prev[3], prev[5], 1)
                    if tt < N_TILES:
                        psg = ps_g.tile([128, NT], F32)
                        psu = ps_u.tile([128, NT], F32)
                        emit_main(wg_sb, psg, x_t, k)
                        # fp8 copies (and their 1-token-shifted twins) of the
                        # pre-conv projections for the taps and next halo;
                        # split across ACT and DVE per branch
                        nc.scalar.activation(
                            out=g_cur[:, k, 0, 3:3 + NT], in_=psg[:],
                            func=AF.Copy, scale=SG8 / SPS)
                        nc.vector.tensor_scalar(
                            g_cur[:, k, 1, 2:2 + NT], psg[:], SG8 / SPS, None,
                            ALU.mult)
                        emit_main(wu_sb, psu, x_t, k)
                        nc.vector.tensor_scalar(
                            u_cur[:, k, 0, 3:3 + NT], psu[:], SG8 / SPS, None,
                            ALU.mult)
                        nc.scalar.activation(
                            out=u_cur[:, k, 1, 2:2 + NT], in_=psu[:],
                            func=AF.Copy, scale=SG8 / SPS)
                    if prev is not None:
                        # silu + h for the previous unit, after this unit's
                        # evacs in the ACT/DVE queues (shortens the
                        # PE-critical evac->tap chain)
                        sg = emit_silu(prev[4])
                        emit_h(prev[2], prev[3], prev[5], sg)
                    if tt < N_TILES:
                        pend[0] = (g_cur, u_cur, h_t, k, psg, psu)
                    if tt >= 1:
                        down_pair(tt - 1, k, last=(tt == N_TILES))
                if tt >= 1:
                    h_tiles.pop(tt - 1)
                if tt < N_TILES:
                    g_prev, u_prev = g_cur, u_cur
                    h_tiles[tt] = h_t

            tile0_phase()
            for tt in range(1, N_TILES + 1):
                if tt + 1 <= N_TILES - 1:
                    load_x(tt + 1)
                tile_phase(tt)

    nc.compile()
    return nc


_NC_CACHE = None


def _get_nc():
    global _NC_CACHE
    if _NC_CACHE is None:
        _NC_CACHE = build_nc()
    return _NC_CACHE


def _split8(a, scale):
    s = np.asarray(a * np.float32(scale), dtype=np.float32)
    hi = s.astype(E4NP)
    lo = (s - hi.astype(np.float32)).astype(E4NP)
    return hi, lo


def _prep_inputs(x, w_gate, w_up, w_down, conv_w):
    xh, xl = _split8(np.ascontiguousarray(x.reshape(TT, D).T), SX)  # [D, TT]
    # compact conv taps [p, fc, branch, k], scaled for the diag stationaries
    cwf = conv_w.reshape(2, NCORES, FC, 128, 4)                 # [br,c,fc,p,k]
    eye = np.eye(128, dtype=np.float16)
    in_maps = []
    for c in range(NCORES):
        fs = slice(c * FC_PER_CORE, (c + 1) * FC_PER_CORE)
        wgh, wgl = _split8(np.ascontiguousarray(w_gate[fs].T), SW)
        wuh, wul = _split8(np.ascontiguousarray(w_up[fs].T), SW)
        wdh, wdl = _split8(np.ascontiguousarray(w_down[:, fs].T), SWD)
        cwc = np.ascontiguousarray(
            cwf[:, c].transpose(2, 1, 0, 3)) * (SPS / SG8)      # [p,fc,br,k]
        in_maps.append({
            "xhT": xh, "xlT": xl, "wghT": wgh, "wglT": wgl,
            "wuhT": wuh, "wulT": wul, "wdhT": wdh, "wdlT": wdl,
            "cw": cwc.astype(np.float32), "eye": eye})
    return in_maps


def run_spmd(in_maps, **kwargs):
    nc = _get_nc()
    return run_bass_kernel_spmd(
        nc, in_maps, core_ids=list(range(NCORES)), **kwargs)


def kernel(x, w_gate, w_up, w_down, conv_w):
    in_maps = _prep_inputs(
        np.asarray(x, dtype=np.float32), np.asarray(w_gate, dtype=np.float32),
        np.asarray(w_up, dtype=np.float32),
        np.asarray(w_down, dtype=np.float32),
        np.asarray(conv_w, dtype=np.float32))
    res = run_spmd(in_maps)
    acc = np.zeros((D, TT), np.float32)
    for r in res.results:
        acc += r["outT"].astype(np.float32)
    return np.ascontiguousarray(acc.T).reshape(B, T, D)


# revision 6
# speedup vs baseline: 1.0229x; 1.0024x over previous
"""CanonGLU feedforward layer on 8 TRN2 NeuronCores — fp8 DoubleRow,
split-operand (hi+lo) version.

Math (per reference):
    gate = x @ w_gate.T ; up = x @ w_up.T            # [B,T,F]
    gate += causal_dconv(gate, conv_w[:F]) ; up += causal_dconv(up, conv_w[F:])
    out  = (up * silu(gate)) @ w_down.T              # [B,T,D]

Sharding: tensor-parallel over d_ff. Core c owns f-slice [c*1024,(c+1)*1024)
of w_gate/w_up/conv_w (column parallel) and w_down (row parallel); x
replicated.  Each core computes a full-shape partial output; the host sums
the 8 partials (the "all-reduce").

Precision scheme: e4m3 DoubleRow matmuls run at 0.5 cyc per output column
with K=256 per instruction — 4x the fp16 rate.  A single e4m3 quantization
of ANY matmul operand exceeds the 2e-2 gate (measured 2.6-3.2e-2 each), so
every operand is split v ~ hi + lo with hi = e4m3(v*S) and lo = e4m3(v*S -
hi) (same scale — no 16x mismatch in the psum), and each matmul becomes
three DoubleRow groups accumulating into one psum:
    x_hi@w_hi + x_hi@w_lo + x_lo@w_hi        (lo*lo term ~2^-8 rel, dropped)
That is 1.5 fp16-equivalents of PE time per matmul = 75% of the fp16
kernel's PE cost, at ~fp16 precision (measured end-to-end 1.08e-2).

The depthwise conv runs ON THE PE as diagonal-stationary DoubleRow matmuls
accumulating into the projection psum: tap k needs c_k[f]*gate[f, t+k-3] =
diag(c_k) @ shift_k(gate8), built from a single-e4m3 copy of the gate
(tap terms are ~10% magnitude; their quantization is in the 1.08e-2 total).

Scales (e4m3 range [2^-9, 240]):
    x*SX (SX=8), w_gate/w_up*SW (SW=512) -> proj psum scale SPS = 4096
    gate8 copy *SG8 (16) -> |gate|<=7.6 -> 122; diag c*SPS/SG8 = c*256 <= 116
    h*SH (SH=4, |h|<=35 -> 140), w_down*SWD (512) -> out psum scale 2048

Engine budget per core (64 (fc,tile) units, TimelineSim cost model):
    PE   ~527us  (mains 2x24 DR + taps 2x2 DR per unit; down 12 DR x 16dc)
    ACT  ~196us  (gate evac, silu, h_hi/h_lo casts, half the down evacs)
    DVE  ~165us  (up evac, h16, h residual, half the down evacs)
    Pool — GPSIMD cannot access PSUM (walrus birverifier); idle.
    DMA  ~130us  (in 29.4MB fp8 hi+lo, out 16.8MB fp16 partials)
"""

import numpy as np
import ml_dtypes

import concourse.bass as bass
import concourse.mybir as mybir
import concourse.tile as tile
from concourse import bacc
from concourse.bass_utils import run_bass_kernel_spmd

F16 = mybir.dt.float16
F32 = mybir.dt.float32
F8 = mybir.dt.float8e4
E4NP = ml_dtypes.float8_e4m3
AF = mybir.ActivationFunctionType
ALU = mybir.AluOpType
DR = mybir.MatmulPerfMode.DoubleRow

B, T, D, F = 2, 2048, 2048, 8192
NCORES = 8
FC_PER_CORE = F // NCORES          # 1024 f per core
TT = B * T                         # 4096 tokens total
NT = 512                           # token tile (one PSUM bank of fp32)
N_TILES = TT // NT                 # 8
TILES_PER_BATCH = T // NT          # 4 (conv halo resets at 0 and 4)
DC = D // 128                      # 16 d-chunks
FC = FC_PER_CORE // 128            # 8 f-chunks per core
GROW = NT + 4                      # conv fp8 row: 3 halo + 512 data + pad
# (pad keeps the slot stride EVEN — an odd-stride DoubleRow moving
# operand crashes the hardware fetch)

SX = 8.0                           # x fp8 scale
SW = 512.0                         # w_gate/w_up fp8 scale
SPS = SX * SW                      # gate/up psum scale (4096)
SG8 = 16.0                         # gate/up fp8 copy scale
SH = 4.0                           # h fp8 scale (|h| <= ~35)
SWD = 512.0                        # w_down fp8 scale
SOUT = SH * SWD                    # down psum scale (2048)


def build_nc():
    nc = bacc.Bacc(None, target_bir_lowering=False, debug=False)

    xhT = nc.dram_tensor("xhT", [D, TT], F8, kind="ExternalInput")
    xlT = nc.dram_tensor("xlT", [D, TT], F8, kind="ExternalInput")
    wghT = nc.dram_tensor("wghT", [D, FC_PER_CORE], F8, kind="ExternalInput")
    wglT = nc.dram_tensor("wglT", [D, FC_PER_CORE], F8, kind="ExternalInput")
    wuhT = nc.dram_tensor("wuhT", [D, FC_PER_CORE], F8, kind="ExternalInput")
    wulT = nc.dram_tensor("wulT", [D, FC_PER_CORE], F8, kind="ExternalInput")
    wdhT = nc.dram_tensor("wdhT", [FC_PER_CORE, D], F8, kind="ExternalInput")
    wdlT = nc.dram_tensor("wdlT", [FC_PER_CORE, D], F8, kind="ExternalInput")
    # compact conv taps [p, fc, branch, k] and a 128x128 identity mask; the
    # diagonal stationary tiles are built on-device (idle DVE at startup).
    cw = nc.dram_tensor("cw", [128, FC, 2, 4], F32, kind="ExternalInput")
    eye = nc.dram_tensor("eye", [128, 128], F16, kind="ExternalInput")
    outT = nc.dram_tensor("outT", [D, TT], F16, kind="ExternalOutput")

    def rearr(t, pat):
        return t.rearrange(pat, p=128)

    xhTr = rearr(xhT, "(dc p) t -> p dc t")
    xlTr = rearr(xlT, "(dc p) t -> p dc t")
    wgTr = [rearr(wghT, "(dc p) f -> p dc f"), rearr(wglT, "(dc p) f -> p dc f")]
    wuTr = [rearr(wuhT, "(dc p) f -> p dc f"), rearr(wulT, "(dc p) f -> p dc f")]
    wdTr = [rearr(wdhT, "(fc p) d -> p fc d"), rearr(wdlT, "(fc p) d -> p fc d")]
    outTr = rearr(outT, "(dp p) t -> p dp t")

    with tile.TileContext(nc) as tc:
        with (
            tc.tile_pool(name="consts", bufs=1) as consts,
            tc.tile_pool(name="xp", bufs=2) as xpool,
            tc.tile_pool(name="gb", bufs=2) as gbpool,
            tc.tile_pool(name="ub", bufs=2) as ubpool,
            tc.tile_pool(name="hp", bufs=2) as hpool,
            tc.tile_pool(name="sgp", bufs=9) as sgpool,
            tc.tile_pool(name="scr", bufs=2) as scrpool,
            tc.tile_pool(name="op", bufs=3) as opool,
            tc.tile_pool(name="psg", bufs=3, space="PSUM") as ps_g,
            tc.tile_pool(name="psu", bufs=2, space="PSUM") as ps_u,
            tc.tile_pool(name="pso", bufs=3, space="PSUM") as ps_o,
        ):
            wg_sb = consts.tile([128, 2, DC, FC_PER_CORE], F8)
            wu_sb = consts.tile([128, 2, DC, FC_PER_CORE], F8)
            wd_sb = consts.tile([128, 2, FC, D], F8)
            cd_sb = consts.tile([128, FC, 2, 2, 2, 128], F8)
            cw_sb = consts.tile([128, FC, 2, 4], F32)
            eye_sb = consts.tile([128, 128], F16)

            x_tiles = {}

            def load_x(tt):
                # one DMA per (tile, half): HWDGE descriptor-gen holds ~627ns
                # per dma_start, so fine-grained loads serialize on it
                x_t = xpool.tile([128, 2, DC, NT], F8)
                nc.sync.dma_start(out=x_t[:, 0], in_=xhTr[:, :, bass.ts(tt, NT)])
                nc.sync.dma_start(out=x_t[:, 1], in_=xlTr[:, :, bass.ts(tt, NT)])
                x_tiles[tt] = x_t

            # PE warmup: dummy matmuls on zeroed SBUF fill the startup DMA
            # wait (w_gate hi+lo 4.2MB + x0 2MB ~ 17us) so the PE p-state
            # reaches (and keeps) 2.4 GHz before the first real matmul.
            warm_sb = consts.tile([128, NT], F16)
            nc.gpsimd.memset(warm_sb[:], 0.0)
            warm_ps = ps_g.tile([128, NT], F32, name="psg")
            for _ in range(20):
                nc.tensor.matmul(
                    warm_ps[:], warm_sb[:, 0:128], warm_sb[:],
                    start=True, stop=True)

            # startup DMA order = first-need order for the tile-0 gate-first
            # schedule.
            nc.sync.dma_start(out=wg_sb[:, 0], in_=wgTr[0][:])
            load_x(0)
            # eye/cw feed the DVE diag build, first needed by the fc0 taps
            # ~20us in — keep them out of the critical w_gate_hi+x(0) prefix
            nc.sync.dma_start(out=eye_sb[:], in_=eye[:])
            nc.sync.dma_start(out=cw_sb[:], in_=cw[:])
            nc.sync.dma_start(out=wg_sb[:, 1], in_=wgTr[1][:])
            nc.sync.dma_start(out=wu_sb[:, 0], in_=wuTr[0][:])
            nc.sync.dma_start(out=wu_sb[:, 1], in_=wuTr[1][:])
            load_x(1)
            nc.sync.dma_start(out=wd_sb[:, 0], in_=wdTr[0][:])
            nc.sync.dma_start(out=wd_sb[:, 1], in_=wdTr[1][:])
            # build the diagonal tap stationaries on the (startup-idle) DVE:
            # cd[:, fc, br, pr, i, :] = eye * c_{2pr+i}[partition]
            for fc in range(FC):
                for br in range(2):
                    for k in range(4):
                        nc.vector.tensor_scalar(
                            cd_sb[:, fc, br, k // 2, k % 2, :], eye_sb[:],
                            cw_sb[:, fc, br, k:k + 1], None, ALU.mult)

            g_prev = u_prev = None
            h_tiles = {}

            def emit_main(w_sb, psp, x_t, k,
                          groups=((0, 0), (1, 0), (0, 1)), start=True):
                # three DoubleRow groups accumulate into one psum; the conv
                # taps close the group later.
                first = start
                for wi, xi in groups:
                    for i in range(DC // 2):
                        nc.tensor.matmul(
                            psp[:],
                            w_sb[:, wi, 2 * i:2 * i + 2, bass.ts(k, 128)],
                            x_t[:, xi, 2 * i:2 * i + 2, :],
                            start=first, stop=False, perf_mode=DR,
                            skip_group_check=not first)
                        first = False

            def emit_taps(buf, k, psp, br):
                # conv taps on the PE: diag(c_k)@shift_k accumulated into the
                # projection psum (residual term is already there).  buf slot
                # 0 holds gate8, slot 1 the 1-token-shifted copy, so the
                # DoubleRow pair (shift 2pr, 2pr+1) is the non-overlapping
                # strided view [p, 2, NT] at offset 2pr — an overlapping AP
                # on the moving operand kills the hardware fetch.
                for pr in range(2):
                    nc.tensor.matmul(
                        psp[:],
                        cd_sb[:, k, br, pr, :, :],
                        buf[:, k, :, 2 * pr:2 * pr + NT],
                        start=False, stop=(pr == 1),
                        perf_mode=DR, skip_group_check=True)

            def emit_silu(psg):
                sg = sgpool.tile([128, NT], F16)
                nc.scalar.activation(
                    out=sg[:], in_=psg[:], func=AF.Silu, scale=1.0 / SPS)
                return sg

            def emit_h(h_t, k, psu, sg):
                # h*SH = (psum_u * SH/SPS) * sg, split hi (fp8) + lo (fp8 of
                # the fp16 residual) for the split down-proj
                h16 = scrpool.tile([128, NT], F16, name="h16")
                nc.vector.scalar_tensor_tensor(
                    out=h16[:], in0=psu[:], scalar=SH / SPS,
                    in1=sg[:], op0=ALU.mult, op1=ALU.mult)
                nc.scalar.activation(
                    out=h_t[:, 0, k, :], in_=h16[:], func=AF.Copy)
                r16 = scrpool.tile([128, NT], F16, name="r16")
                nc.vector.tensor_tensor(
                    out=r16[:], in0=h16[:], in1=h_t[:, 0, k, :], op=ALU.subtract)
                nc.scalar.activation(
                    out=h_t[:, 1, k, :], in_=r16[:], func=AF.Copy)

            def down_pair(tt, k, last=False):
                """Down-proj for dc = 2k, 2k+1: three DoubleRow groups per
                psum (hh, hl, lh), evacs alternated ACT/DVE (GPSIMD cannot
                read PSUM), one out-DMA per pair."""
                h_t = h_tiles[tt]
                o_sb = opool.tile([128, 2, NT], F16)
                for half in range(2):
                    dc = 2 * k + half
                    pool, tag = ((ps_o, "pso"), (ps_g, "psg"),
                                 (ps_u, "psu"))[dc % 3 if last else 0]
                    pso = pool.tile([128, NT], F32, name=tag)
                    first = True
                    for wi, hi in ((0, 0), (1, 0), (0, 1)):
                        for j in range(FC // 2):
                            nc.tensor.matmul(
                                pso[:],
                                wd_sb[:, wi, 2 * j:2 * j + 2, bass.ts(dc, 128)],
                                h_t[:, hi, 2 * j:2 * j + 2, :],
                                start=first,
                                stop=(wi == 0 and hi == 1 and j == FC // 2 - 1),
                                perf_mode=DR, skip_group_check=not first)
                            first = False
                    o_slot = o_sb[:, half, :]
                    if dc % 2 == 0:
                        nc.scalar.activation(
                            out=o_slot, in_=pso[:], func=AF.Copy,
                            scale=1.0 / SOUT)
                    else:
                        nc.vector.tensor_scalar(
                            o_slot, pso[:], 1.0 / SOUT, None, ALU.mult)
                if last:
                    # tail drain: per-dc DMAs so the final transfer only
                    # waits its own evac
                    for half in range(2):
                        eng = nc.sync if half == 0 else nc.scalar
                        eng.dma_start(
                            out=outTr[:, 2 * k + half, bass.ts(tt, NT)],
                            in_=o_sb[:, half, :])
                else:
                    eng = nc.sync if k % 2 == 0 else nc.scalar
                    eng.dma_start(
                        out=outTr[:, 2 * k:2 * k + 2, bass.ts(tt, NT)],
                        in_=o_sb[:])

            def new_bufs(tt):
                # slot 0: gate8/up8 at token offset -3 (3 halo + 512 data);
                # slot 1: the same data shifted one token (2 halo + 512 data)
                g_cur = gbpool.tile([128, FC, 2, GROW], F8)
                u_cur = ubpool.tile([128, FC, 2, GROW], F8)
                h_t = hpool.tile([128, 2, FC, NT], F8)
                # conv halo: last tokens of the previous tile's fp8 copies
                # (zeros at the start of each batch — causal left pad).
                for br, (buf, prev) in enumerate(((g_cur, g_prev),
                                                  (u_cur, u_prev))):
                    if tt % TILES_PER_BATCH == 0:
                        nc.vector.memset(buf[:, :, 0, 0:3], 0.0)
                        nc.vector.memset(buf[:, :, 1, 0:2], 0.0)
                    else:
                        nc.vector.tensor_copy(
                            out=buf[:, :, 0, 0:3],
                            in_=prev[:, :, 0, NT:NT + 3])
                        nc.vector.tensor_copy(
                            out=buf[:, :, 1, 0:2],
                            in_=prev[:, :, 1, NT:NT + 2])
                return g_cur, u_cur, h_t

            def tile0_phase():
                """Tile 0 streams gate first (only w_gate + x(0) must have
                landed), then up — hides the w_up DMA behind the gate pass
                instead of stalling the PE."""
                nonlocal g_prev, u_prev
                x_t = x_tiles.pop(0)
                g_cur, u_cur, h_t = new_bufs(0)
                sgs = {}
                # gate taps lag TWO mains so the first tap (which needs the
                # x_lo DMA-built diags) never stalls the in-order PE queue
                # startup waves: while w_gate_lo is in flight, run the
                # groups needing only w_gate_hi + x(0) for the first three
                # fc units (all three psg banks)
                early = {}
                epool = [(ps_g, "psg")] * 3 + [(ps_u, "psu")] * 2 +                     [(ps_o, "pso")]
                for k in range(6):
                    pool, tag = epool[k]
                    psg = pool.tile([128, NT], F32, name=tag)
                    emit_main(wg_sb, psg, x_t, k, groups=((0, 0),))
                    early[k] = psg
                for k in range(6):
                    emit_main(wg_sb, early[k], x_t, k, groups=((0, 1),),
                              start=False)
                gq = []
                for k in range(FC):
                    if len(gq) == 2:
                        kk, psg_old = gq.pop(0)
                        emit_taps(g_cur, kk, psg_old, 0)
                        sgs[kk] = emit_silu(psg_old)
                    if k in early:
                        psg = early.pop(k)
                        emit_main(wg_sb, psg, x_t, k, groups=((1, 0),),
                                  start=False)
                    else:
                        psg = ps_g.tile([128, NT], F32, name="psg")
                        emit_main(wg_sb, psg, x_t, k)
                    nc.scalar.activation(
                        out=g_cur[:, k, 0, 3:3 + NT], in_=psg[:],
                        func=AF.Copy, scale=SG8 / SPS)
                    nc.vector.tensor_scalar(
                        g_cur[:, k, 1, 2:2 + NT], psg[:], SG8 / SPS, None,
                        ALU.mult)
                    gq.append((k, psg))
                for kk, psg_old in gq:
                    emit_taps(g_cur, kk, psg_old, 0)
                    sgs[kk] = emit_silu(psg_old)
                upend = None
                for k in range(FC):
                    if upend is not None:
                        emit_taps(u_cur, k - 1, upend, 1)
                        emit_h(h_t, k - 1, upend, sgs[k - 1])
                    psu = ps_u.tile([128, NT], F32)
                    emit_main(wu_sb, psu, x_t, k)
                    nc.vector.tensor_scalar(
                        u_cur[:, k, 0, 3:3 + NT], psu[:], SG8 / SPS, None,
                        ALU.mult)
                    nc.scalar.activation(
                        out=u_cur[:, k, 1, 2:2 + NT], in_=psu[:],
                        func=AF.Copy, scale=SG8 / SPS)
                    upend = psu
                emit_taps(u_cur, FC - 1, upend, 1)
                emit_h(h_t, FC - 1, upend, sgs[FC - 1])
                g_prev, u_prev = g_cur, u_cur
                h_tiles[0] = h_t

            # pending (g_cur, u_cur, h_t, fc, psg, psu): projections whose
            # conv taps + silu + h are emitted one fc later, so the in-order
            # PE queue never waits on the evac->tap dependency.
            pend = [None]

            def tile_phase(tt):
                """Emit gate/up(tt) interleaved with down(tt-1)."""
                nonlocal g_prev, u_prev
                if tt < N_TILES:
                    x_t = x_tiles.pop(tt)
                    g_cur, u_cur, h_t = new_bufs(tt)
                for k in range(FC):
                    prev = pend[0]
                    pend[0] = None
                    if prev is not None:
                        # PE: taps for the previous unit first (evacs landed
                        # last iteration)
                        emit_taps(prev[0], prev[3], prev[4], 0)
                        emit_taps(prev[1], prev[3], prev[5], 1)
                    if tt < N_TILES:
                        psg = ps_g.tile([128, NT], F32)
                        psu = ps_u.tile([128, NT], F32)
                        emit_main(wg_sb, psg, x_t, k)
                        # fp8 copies (and their 1-token-shifted twins) of the
                        # pre-conv projections for the taps and next halo;
                        # split across ACT and DVE per branch
                        nc.scalar.activation(
                            out=g_cur[:, k, 0, 3:3 + NT], in_=psg[:],
                            func=AF.Copy, scale=SG8 / SPS)
                        nc.vector.tensor_scalar(
                            g_cur[:, k, 1, 2:2 + NT], psg[:], SG8 / SPS, None,
                            ALU.mult)
                        emit_main(wu_sb, psu, x_t, k)
                        nc.vector.tensor_scalar(
                            u_cur[:, k, 0, 3:3 + NT], psu[:], SG8 / SPS, None,
                            ALU.mult)
                        nc.scalar.activation(
                            out=u_cur[:, k, 1, 2:2 + NT], in_=psu[:],
                            func=AF.Copy, scale=SG8 / SPS)
                    if prev is not None:
                        # silu + h for the previous unit, after this unit's
                        # evacs in the ACT/DVE queues (shortens the
                        # PE-critical evac->tap chain)
                        sg = emit_silu(prev[4])
                        emit_h(prev[2], prev[3], prev[5], sg)
                    if tt < N_TILES:
                        pend[0] = (g_cur, u_cur, h_t, k, psg, psu)
                    if tt >= 1:
                        down_pair(tt - 1, k, last=(tt == N_TILES))
                if tt >= 1:
                    h_tiles.pop(tt - 1)
                if tt < N_TILES:
                    g_prev, u_prev = g_cur, u_cur
                    h_tiles[tt] = h_t

            tile0_phase()
            for tt in range(1, N_TILES + 1):
                if tt + 1 <= N_TILES - 1:
                    load_x(tt + 1)
                tile_phase(tt)

    nc.compile()
    return nc


_NC_CACHE = None


def _get_nc():
    global _NC_CACHE
    if _NC_CACHE is None:
        _NC_CACHE = build_nc()
    return _NC_CACHE


def _split8(a, scale):
    s = np.asarray(a * np.float32(scale), dtype=np.float32)
    hi = s.astype(E4NP)
    lo = (s - hi.astype(np.float32)).astype(E4NP)
    return hi, lo


def _prep_inputs(x, w_gate, w_up, w_down, conv_w):
    xh, xl = _split8(np.ascontiguousarray(x.reshape(TT, D).T), SX)  # [D, TT]
    # compact conv taps [p, fc, branch, k], scaled for the diag stationaries
    cwf = conv_w.reshape(2, NCORES, FC, 128, 4)                 # [br,c,fc,p,k]
    eye = np.eye(128, dtype=np.float16)
    in_maps = []
    for c in range(NCORES):
        fs = slice(c * FC_PER_CORE, (c + 1) * FC_PER_CORE)
        wgh, wgl = _split8(np.ascontiguousarray(w_gate[fs].T), SW)
        wuh, wul = _split8(np.ascontiguousarray(w_up[fs].T), SW)
        wdh, wdl = _split8(np.ascontiguousarray(w_down[:, fs].T), SWD)
        cwc = np.ascontiguousarray(
            cwf[:, c].transpose(2, 1, 0, 3)) * (SPS / SG8)      # [p,fc,br,k]
        in_maps.append({
            "xhT": xh, "xlT": xl, "wghT": wgh, "wglT": wgl,
            "wuhT": wuh, "wulT": wul, "wdhT": wdh, "wdlT": wdl,
            "cw": cwc.astype(np.float32), "eye": eye})
    return in_maps


def run_spmd(in_maps, **kwargs):
    nc = _get_nc()
    return run_bass_kernel_spmd(
        nc, in_maps, core_ids=list(range(NCORES)), **kwargs)


def kernel(x, w_gate, w_up, w_down, conv_w):
    in_maps = _prep_inputs(
        np.asarray(x, dtype=np.float32), np.asarray(w_gate, dtype=np.float32),
        np.asarray(w_up, dtype=np.float32),
        np.asarray(w_down, dtype=np.float32),
        np.asarray(conv_w, dtype=np.float32))
    res = run_spmd(in_maps)
    acc = np.zeros((D, TT), np.float32)
    for r in res.results:
        acc += r["outT"].astype(np.float32)
    return np.ascontiguousarray(acc.T).reshape(B, T, D)
